# revision 27
# baseline (speedup 1.0000x reference)
"""GCN 2-layer kernel for trn2: host preprocessing + Bass kernel builder.

Math (per GCNConv, PyG-style):
  out = D^-1/2 (A+I) D^-1/2 (X W) + b
Layer1 -> relu -> Layer2.

Device plan (8 cores, SPMD), v2:
  P1: h1' = dinv .* (x_shard @ W1)             (ID-sharded)
  AG1: allgather h1' -> table [NP, HID] bf16   (row r = h1' of node id r)
  P3: per group of G=7 dst blocks: 4 batched dma_gathers (one per src
      quadrant), shared one-hot indicator built once per (group, quadrant),
      per-block indicator matmuls -> psum [HID, 128] (transposed), dinv_dst
      scale (DVE), relu+b1 (ACT), @W2 -> [128, 48], dinv scale -> h2'
  AG2: allgather h2' (128-col padded rows, 48 used) -> [NP, 128] bf16
  P5: per group: 4 batched 96B-payload gathers (48 cols from 256B-pitch
      rows), indicator matmuls -> psum [128, 48], dinv scale + b2 -> out.
Self-loops are regular edges in BOTH layers. Node->position assignment is
quadrant-aligned (pos//QS == id//QS) so both layers share one edge-chunk
structure (same dl/indicator streams; only gather index values differ).
Host: unpermute rows, slice [:N0, :CLS].
"""

from dataclasses import dataclass

import numpy as np

import concourse.bass as bass
import concourse.mybir as mybir
import concourse.tile as tile
from concourse import bacc

FP = mybir.dt.float32
BF = mybir.dt.bfloat16
IND_BUFS = 6


@dataclass
class Cfg:
    N0: int = 100000     # real nodes
    W: int = 8           # cores
    SHARD: int = 12544   # nodes per core (multiple of 128)
    F: int = 512         # in features
    HID: int = 128
    CLS: int = 40
    CPAD: int = 48       # padded class dim (48*2B = 96B gather payload)
    Q: int = 4           # src quadrants (int16 gather index limit)
    G: int = 7           # dst blocks per gather group

    @property
    def NP(self):
        return self.W * self.SHARD

    @property
    def QS(self):
        return self.NP // self.Q

    @property
    def NB(self):
        return self.SHARD // 128  # blocks per core (98)

    @property
    def NG(self):
        return self.NB // self.G  # groups per core (14)


@dataclass
class Meta:
    kq: np.ndarray = None        # [NG, Q, G] chunks per (group, quadrant, block)
    node_of_pos: np.ndarray = None  # [W, SHARD]


def preprocess(cfg: Cfg, x, edge_index, W1, b1, W2, b2):
    N0, W, SHARD, NP, QS = cfg.N0, cfg.W, cfg.SHARD, cfg.NP, cfg.QS
    NB, Q, G, NG = cfg.NB, cfg.Q, cfg.G, cfg.NG
    x = np.asarray(x, dtype=np.float32)
    edge_index = np.asarray(edge_index)
    W1 = np.asarray(W1, np.float32)
    b1 = np.asarray(b1, np.float32)
    W2 = np.asarray(W2, np.float32)
    b2 = np.asarray(b2, np.float32)

    s = edge_index[0].astype(np.int64)
    d = edge_index[1].astype(np.int64)
    loops = np.arange(N0, dtype=np.int64)
    s_all = np.concatenate([s, loops])
    d_all = np.concatenate([d, loops])

    deg = np.bincount(d_all, minlength=NP).astype(np.float64)
    with np.errstate(divide="ignore"):
        dinv = np.where(deg > 0, 1.0 / np.sqrt(deg), 0.0).astype(np.float32)

    # --- degree-balanced position assignment (global serpentine) ---
    nblk = W * NB
    order = np.argsort(-deg[:NP], kind="stable")
    r = np.arange(NP, dtype=np.int64)
    cyc = r % (2 * nblk)
    blk = np.where(cyc < nblk, cyc, 2 * nblk - 1 - cyc)  # serpentine
    slot_ctr = r // (2 * nblk) * 2 + (cyc >= nblk).astype(np.int64)
    pos_of_node = np.empty(NP, dtype=np.int64)
    pos_of_node[order] = (blk % W) * SHARD + (blk // W) * 128 + slot_ctr
    node_of_pos = np.empty(NP, dtype=np.int64)
    node_of_pos[pos_of_node] = np.arange(NP, dtype=np.int64)

    # --- edge routing (self-loops handled as on-device diagonal) ---
    dst_pos = pos_of_node[d]
    src_pos = pos_of_node[s]
    q_e = src_pos // QS

    core_e = dst_pos // SHARD
    blk_e = (dst_pos % SHARD) // 128
    slot_e = dst_pos % 128
    g_e = blk_e // G
    bbg_e = blk_e % G

    # segment key in (core, g, q, bb) order
    key = ((core_e * NG + g_e) * Q + q_e) * G + bbg_e
    nseg = W * NG * Q * G
    counts = np.bincount(key, minlength=nseg).reshape(W, NG, Q, G)
    kq = np.ceil(counts.max(axis=0) / 128.0).astype(np.int64)  # [NG, Q, G]

    seg_pad = kq * 128  # [NG, Q, G] edges incl. padding
    seg_off = np.zeros(nseg // W, dtype=np.int64)
    seg_off[1:] = np.cumsum(seg_pad.reshape(-1))[:-1]
    seg_off = seg_off.reshape(NG, Q, G)
    EPAD = int(seg_pad.sum())
    CT = EPAD // 128  # total chunks

    # rank of each edge within its (core,g,q,bb) segment
    order = np.argsort(key, kind="stable")
    key_s = key[order]
    seg_start = np.zeros(nseg + 1, dtype=np.int64)
    seg_start[1:] = np.cumsum(counts.reshape(-1))
    rank = np.arange(len(key_s), dtype=np.int64) - seg_start[key_s]
    # target slot in the per-core padded stream
    tgt = seg_off.reshape(-1)[key_s % (NG * Q * G)] + rank
    core_s = key_s // (NG * Q * G)

    idx_stream = np.zeros((W, EPAD), dtype=np.int16)
    dl_stream = np.full((W, EPAD), -1.0, dtype=np.float32)
    flat = core_s * EPAD + tgt
    idx_stream.reshape(-1)[flat] = (src_pos - q_e * QS)[order].astype(np.int16)
    dl_stream.reshape(-1)[flat] = slot_e[order].astype(np.float32)

    # wrap idx into gather layout: within each (g,q) call's local stream,
    # local index i -> [i % 16, callcol0*8 + i // 16], tiled x8 to 128 parts.
    # call (g,q) spans segments (g,q,0..G-1): local wrap == global wrap as
    # long as call start offsets are multiples of 16 (they are: x128).
    def wrap_idx(stream):
        # stream [W, EPAD]; global position within call cg0*128.. ; since
        # every call boundary is 128-aligned and wrap is (i%16, i//16), a
        # single global wrap per 16 works iff done call-locally. Call starts
        # are 128-aligned => i_local%16 == i_global%16 and
        # i_local//16 == i_global//16 - call0*8. So one global reshape works.
        a = stream.reshape(W, CT, 8, 16)
        wrapped = a.transpose(0, 3, 1, 2).reshape(W, 16, CT * 8)
        return np.tile(wrapped, (1, 8, 1)).astype(np.int16)

    idx_t = wrap_idx(idx_stream)
    dl_t = dl_stream.reshape(W, CT, 128).transpose(0, 2, 1)  # [W,128,CT]

    import ml_dtypes
    bft = ml_dtypes.bfloat16

    dinv_pos = dinv[node_of_pos.reshape(W, SHARD)]  # [W, SHARD] pos-order

    per_core = []
    for c in range(W):
        # host-gathers x rows into position order (pads -> zero rows)
        nop = node_of_pos[c * SHARD:(c + 1) * SHARD]
        xs = np.zeros((SHARD, cfg.F), np.float32)
        real = nop < N0
        xs[real] = x[nop[real]]
        inp = {
            "xT": np.ascontiguousarray(xs.T).astype(bft),          # [F, SHARD]
            "w1": W1.astype(bft),                                  # [F, HID]
            "b1col": b1.reshape(cfg.HID, 1).copy(),                # [HID, 1]
            "w2p": np.pad(W2, ((0, 0), (0, cfg.CPAD - cfg.CLS))).astype(bft),
            "b2rep": np.broadcast_to(
                np.pad(b2, (0, cfg.CPAD - cfg.CLS)), (128, cfg.CPAD)).copy(),
            "iota": np.broadcast_to(
                np.arange(128, dtype=np.float32), (128, 128)).astype(bft).copy(),
            "ident": np.eye(128, dtype=np.float32).astype(bft),
            "idx": idx_t[c],
            "dl": dl_t[c].astype(bft),
            # pos-order scale: col bb = dinv at positions of block bb
            "dinv_pc": dinv_pos[c].reshape(cfg.NB, 128).T.copy(),  # [128, NB]
            # pos-order row for dst scaling in [HID, slots] orientation
            "dinv_pr": np.broadcast_to(dinv_pos[c], (128, SHARD)).copy(),
        }
        per_core.append(inp)

    meta = Meta(kq=kq, node_of_pos=node_of_pos.reshape(W, SHARD))
    return per_core, meta, dinv


def postprocess(cfg: Cfg, outs, meta: Meta):
    """outs: list of [SHARD, CPAD] per core -> [N0, CLS] in node order."""
    res = np.zeros((cfg.NP, cfg.CPAD), np.float32)
    for c in range(cfg.W):
        res[meta.node_of_pos[c]] = outs[c]
    return res[:cfg.N0, :cfg.CLS]


def dma_gather_narrow(nc, out_ap, in_ap, idxs_ap, num_idxs, elem_size,
                      elem_step, queue_num):
    """dma_gather with elem_size_bytes < 256 (payload narrower than the 256B
    row pitch). Mirrors BassGpSimd.dma_gather's non-transpose DRAM path minus
    the %256 payload assert (ucode only requires the *stride* be a multiple
    of 256B; payload is free)."""
    eng = nc.gpsimd
    assert idxs_ap.dtype == mybir.dt.int16
    esz = elem_size * mybir.dt.size(in_ap.dtype)
    assert esz > 0 and esz % 2 == 0
    stride_bytes = elem_step * mybir.dt.size(in_ap.dtype)
    assert stride_bytes % 256 == 0
    stride_bytes_256 = stride_bytes // 256
    assert stride_bytes_256 < 256
    assert in_ap.ap[0][0] == elem_step
    assert in_ap.ap[-1][1] == out_ap.ap[-1][1] == elem_size
    assert out_ap.ap[0][1] * out_ap.ap[1][1] == num_idxs
    _in_ap = eng.lower_ap_dma(in_ap, for_custom_bir_dma=True)
    _idxs_ap = eng.lower_ap(idxs_ap)
    _out_ap = eng.lower_ap(out_ap)
    return eng.add_instruction(
        mybir.InstDMAGatherAnt(
            name=eng.bass.get_next_instruction_name(),
            ins=[*_in_ap, _idxs_ap, eng.lower_val_access(eng.to_reg(num_idxs))],
            outs=[_out_ap],
            transpose=False,
            num_idxs=num_idxs,
            elem_size=elem_size,
            stride_bytes_256=stride_bytes_256,
            gen_mode=0,
            single_packet=False,
            queue_num=queue_num,
            sbuf_tokens_per_rank=0,
            sbuf_free_dim_per_rank=0,
            sbuf_free_dim_pad_per_rank=0,
            sbuf_byte_offset=0,
        )
    )


def build(cfg: Cfg, meta: Meta):
    W, SHARD, NP, F, HID, CPAD = cfg.W, cfg.SHARD, cfg.NP, cfg.F, cfg.HID, cfg.CPAD
    NB, Q, QS, G, NG = cfg.NB, cfg.Q, cfg.QS, cfg.G, cfg.NG
    kq = meta.kq  # [NG, Q, G]
    # chunk bookkeeping (all compile-time)
    ch_gqb = kq  # chunks per (g,q,bb)
    ch_gq = kq.sum(axis=2)          # [NG, Q]
    ch_g = ch_gq.sum(axis=1)        # [NG]
    CT = int(ch_g.sum())
    CHG = int(ch_g.max())           # chunks per group (max)
    CHGQ = int(ch_gq.max())
    g_off = np.zeros(NG + 1, dtype=np.int64)
    g_off[1:] = np.cumsum(ch_g)
    KT = F // 128

    nc = bacc.Bacc("TRN2", target_bir_lowering=False, debug=False,
                   num_devices=W, num_swdge_queues=4)

    xT = nc.dram_tensor("xT", [F, SHARD], BF, kind="ExternalInput")
    w1 = nc.dram_tensor("w1", [F, HID], BF, kind="ExternalInput")
    b1col = nc.dram_tensor("b1col", [HID, 1], FP, kind="ExternalInput")
    w2p = nc.dram_tensor("w2p", [HID, CPAD], BF, kind="ExternalInput")
    b2rep = nc.dram_tensor("b2rep", [128, CPAD], FP, kind="ExternalInput")
    iota = nc.dram_tensor("iota", [128, 128], BF, kind="ExternalInput")
    ident = nc.dram_tensor("ident", [128, 128], BF, kind="ExternalInput")
    idx = nc.dram_tensor("idx", [128, CT * 8], mybir.dt.int16, kind="ExternalInput")
    dl = nc.dram_tensor("dl", [128, CT], BF, kind="ExternalInput")
    dinv_pc = nc.dram_tensor("dinv_pc", [128, NB], FP, kind="ExternalInput")
    dinv_pr = nc.dram_tensor("dinv_pr", [128, SHARD], FP, kind="ExternalInput")
    out_s = nc.dram_tensor("out_s", [SHARD, CPAD], FP, kind="ExternalOutput")

    ag1_in = nc.dram_tensor("ag1_in", [SHARD, HID], BF)
    ag1_out = nc.dram_tensor("ag1_out", [NP, HID], BF, addr_space="Shared")
    # L2 table: 256B-pitch rows, only first CPAD cols used
    ag2_in = nc.dram_tensor("ag2_in", [SHARD, 128], BF)
    ag2_out = nc.dram_tensor("ag2_out", [NP, 128], BF, addr_space="Shared")

    qctr = [0]

    def next_q():
        qctr[0] = (qctr[0] + 1) % 4
        return qctr[0]

    with tile.TileContext(nc) as tc:
        with (
            tc.tile_pool(name="const", bufs=1) as cpool,
            tc.tile_pool(name="p1", bufs=4) as p1pool,
            tc.tile_pool(name="meta1", bufs=2) as mpool,
            tc.tile_pool(name="gath1", bufs=2) as g1pool,
            tc.tile_pool(name="gath2", bufs=2) as g2pool,
            tc.tile_pool(name="indp", bufs=IND_BUFS) as ipool,
            tc.tile_pool(name="mid", bufs=3) as midpool,
            tc.tile_pool(name="psa", bufs=2, space="PSUM") as psapool,
            tc.tile_pool(name="psb", bufs=2, space="PSUM") as psbpool,
        ):
            # ---- constants ----
            iota_t = cpool.tile([128, 128], BF)
            nc.sync.dma_start(out=iota_t[:, :], in_=iota[:, :])
            ident_t = cpool.tile([128, 128], BF)
            nc.sync.dma_start(out=ident_t[:, :], in_=ident[:, :])
            b1_t = cpool.tile([HID, 1], FP)
            nc.sync.dma_start(out=b1_t[:, :], in_=b1col[:, :])
            w2_t = cpool.tile([HID, CPAD], BF)
            nc.sync.dma_start(out=w2_t[:, :], in_=w2p[:, :])
            b2_t = cpool.tile([128, CPAD], FP)
            nc.sync.dma_start(out=b2_t[:, :], in_=b2rep[:, :])
            dp_t = cpool.tile([128, NB], FP)
            nc.sync.dma_start(out=dp_t[:, :], in_=dinv_pc[:, :])
            w1k_t = cpool.tile([128, KT, HID], BF)
            for k in range(KT):
                nc.sync.dma_start(out=w1k_t[:, k, :], in_=w1[k * 128:(k + 1) * 128, :])

            # ---- phase 1: h1' = dinv .* (x @ W1) ----
            for t in range(NB):
                psh = psapool.tile([128, HID], FP, space="PSUM", tag="ph1")
                for k in range(KT):
                    xt_t = p1pool.tile([128, 128], BF, tag="xt")
                    nc.sync.dma_start(
                        out=xt_t[:, :],
                        in_=xT[k * 128:(k + 1) * 128, t * 128:(t + 1) * 128])
                    nc.tensor.matmul(out=psh[:, :], lhsT=xt_t[:, :],
                                     rhs=w1k_t[:, k, :],
                                     start=(k == 0), stop=(k == KT - 1))
                h1p = p1pool.tile([128, HID], BF, tag="h1p")
                nc.scalar.activation(out=h1p[:, :], in_=psh[:, :],
                                     func=mybir.ActivationFunctionType.Copy,
                                     scale=dp_t[:, t:t + 1])
                nc.scalar.dma_start(out=ag1_in[t * 128:(t + 1) * 128, :],
                                    in_=h1p[:, :])

            # ---- AG1 ----
            nc.gpsimd.collective_compute(
                "AllGather", mybir.AluOpType.bypass,
                replica_groups=[list(range(W))],
                ins=[ag1_in[:, :]], outs=[ag1_out[:, :]],
            )

            # ================= phase 3: L1 agg + relu + @W2 =================
            def load_group_meta(g, idx_dram):
                c0, c1 = int(g_off[g]), int(g_off[g + 1])
                ncch = c1 - c0
                ixt = mpool.tile([128, CHG * 8], mybir.dt.int16, tag="ix",
                                 bufs=3)
                nc.sync.dma_start(out=ixt[:, :ncch * 8],
                                  in_=idx_dram[:, c0 * 8:c1 * 8])
                dlt = mpool.tile([128, CHG], BF, tag="dl", bufs=3)
                nc.sync.dma_start(out=dlt[:, :ncch], in_=dl[:, c0:c1])
                return ixt, dlt

            def build_inds(g, dlt):
                """one indicator tile per (g, q) spanning that q's chunks"""
                inds = []
                co = 0
                for q in range(Q):
                    nq = int(ch_gq[g, q])
                    if nq == 0:
                        inds.append(None)
                        continue
                    ind = ipool.tile([128, CHGQ, 128], BF, tag="ind")
                    nc.vector.tensor_tensor(
                        out=ind[:, :nq, :],
                        in0=dlt[:, co:co + nq].to_broadcast([128, nq, 128]),
                        in1=iota_t[:, None, :].to_broadcast([128, nq, 128]),
                        op=mybir.AluOpType.is_equal,
                    )
                    inds.append(ind)
                    co += nq
                return inds

            # calls capped at CAPCH chunks (2048 idxs = 128 descs/engine) so a
            # whole call fits the SDMA descriptor ring without Q7 busy-waiting
            CAPCH = 16

            def gather_gq(g, q, gq_tile, ixt, table, width, narrow):
                """gather quadrant q of group g into its own tile"""
                co = int(ch_gq[g, :q].sum())
                nq = int(ch_gq[g, q])
                s = 0
                while s < nq:
                    n = min(CAPCH, nq - s)
                    if narrow:
                        dma_gather_narrow(
                            nc, gq_tile[:, s:s + n, :],
                            table[q * QS:(q + 1) * QS, :width],
                            ixt[:, (co + s) * 8:(co + s + n) * 8],
                            n * 128, width, 128, next_q())
                    else:
                        nc.gpsimd.dma_gather(
                            gq_tile[:, s:s + n, :],
                            table[q * QS:(q + 1) * QS, :],
                            ixt[:, (co + s) * 8:(co + s + n) * 8],
                            n * 128, n * 128, width,
                            single_packet=False, queue_num=next_q(),
                        )
                    s += n

            def gather_group(g, pool, ixt, table, width, narrow, tag):
                tiles = []
                for q in range(Q):
                    gq_tile = pool.tile([128, CHGQ, width], BF, tag=tag,
                                        bufs=8)
                    gather_gq(g, q, gq_tile, ixt, table, width, narrow)
                    tiles.append(gq_tile)
                return tiles

            def load_drt(g):
                drt = mpool.tile([128, G * 128], FP, tag="drt", bufs=3)
                nc.sync.dma_start(
                    out=drt[:, :],
                    in_=dinv_pr[:, g * G * 128:(g + 1) * G * 128])
                return drt

            def load_h1ds(g):
                hs = []
                for bb in range(G):
                    blk = g * G + bb
                    h1d = mpool.tile([128, HID], BF, tag="h1d", bufs=16)
                    nc.sync.dma_start(
                        out=h1d[:, :],
                        in_=ag1_in[blk * 128:(blk + 1) * 128, :])
                    hs.append(h1d)
                return hs

            # prologue: two-group lookahead of metadata (flows during AG1)
            p3_meta = {0: load_group_meta(0, idx), 1: load_group_meta(1, idx)}
            p3_inds = {0: build_inds(0, p3_meta[0][1])}
            p3_drt = {0: load_drt(0)}
            p3_h1d = {0: load_h1ds(0)}

            for g in range(NG):
                gqs = gather_group(g, g1pool, p3_meta[g][0], ag1_out, HID,
                                   False, "g1")
                # prefetch: meta two ahead, inds/drt/diag one ahead
                if g + 2 < NG:
                    p3_meta[g + 2] = load_group_meta(g + 2, idx)
                if g + 1 < NG:
                    p3_inds[g + 1] = build_inds(g + 1, p3_meta[g + 1][1])
                    p3_drt[g + 1] = load_drt(g + 1)
                    p3_h1d[g + 1] = load_h1ds(g + 1)

                inds = p3_inds[g]
                drt = p3_drt[g]
                h1ds = p3_h1d[g]
                for bb in range(G):
                    blk = g * G + bb
                    ps1 = psapool.tile([128, 128], FP, space="PSUM", tag="ps1")
                    nmm = int(ch_gqb[g, :, bb].sum())
                    # self-loop diagonal: ps1[:, j] += h1'[slot j] (transpose
                    # via identity matmul; dst dinv applied later via drt)
                    nc.tensor.matmul(out=ps1[:, :], lhsT=h1ds[bb][:, :],
                                     rhs=ident_t[:, :],
                                     start=True, stop=(nmm == 0))
                    done = 0
                    for q in range(Q):
                        b0 = int(ch_gqb[g, q, :bb].sum())
                        for ck in range(int(ch_gqb[g, q, bb])):
                            nc.tensor.matmul(
                                out=ps1[:, :],
                                lhsT=gqs[q][:, b0 + ck, :],
                                rhs=inds[q][:, b0 + ck, :],
                                start=False, stop=(done == nmm - 1))
                            done += 1
                    t1 = midpool.tile([128, 128], FP, tag="t1")
                    nc.vector.tensor_tensor(
                        out=t1[:, :], in0=ps1[:, :],
                        in1=drt[:, bb * 128:(bb + 1) * 128],
                        op=mybir.AluOpType.mult)
                    r1 = midpool.tile([128, 128], BF, tag="r1")
                    nc.scalar.activation(out=r1[:, :], in_=t1[:, :],
                                         func=mybir.ActivationFunctionType.Relu,
                                         bias=b1_t[:, :1])
                    ps2 = psbpool.tile([128, CPAD], FP, space="PSUM", tag="ps2")
                    nc.tensor.matmul(out=ps2[:, :], lhsT=r1[:, :], rhs=w2_t[:, :],
                                     start=True, stop=True)
                    h2p = midpool.tile([128, CPAD], BF, tag="h2p")
                    nc.scalar.activation(out=h2p[:, :], in_=ps2[:, :],
                                         func=mybir.ActivationFunctionType.Copy,
                                         scale=dp_t[:, blk:blk + 1])
                    nc.scalar.dma_start(
                        out=ag2_in[blk * 128:(blk + 1) * 128, :CPAD],
                        in_=h2p[:, :])

            # ---- AG2 ----
            nc.gpsimd.collective_compute(
                "AllGather", mybir.AluOpType.bypass,
                replica_groups=[list(range(W))],
                ins=[ag2_in[:, :]], outs=[ag2_out[:, :]],
            )

            # ================= phase 5: L2 agg + b2 -> out =================
            def load_h2ds(g):
                hs = []
                for bb in range(G):
                    blk = g * G + bb
                    h2d = mpool.tile([128, CPAD], BF, tag="h2d", bufs=16)
                    nc.sync.dma_start(
                        out=h2d[:, :],
                        in_=ag2_in[blk * 128:(blk + 1) * 128, :CPAD])
                    hs.append(h2d)
                return hs

            p5_meta = {0: load_group_meta(0, idx), 1: load_group_meta(1, idx)}
            p5_inds = {0: build_inds(0, p5_meta[0][1])}
            p5_h2d = {0: load_h2ds(0)}

            for g in range(NG):
                gqs = gather_group(g, g2pool, p5_meta[g][0], ag2_out, CPAD,
                                   True, "g2")
                if g + 2 < NG:
                    p5_meta[g + 2] = load_group_meta(g + 2, idx)
                if g + 1 < NG:
                    p5_inds[g + 1] = build_inds(g + 1, p5_meta[g + 1][1])
                    p5_h2d[g + 1] = load_h2ds(g + 1)

                inds = p5_inds[g]
                h2ds = p5_h2d[g]
                for bb in range(G):
                    blk = g * G + bb
                    ps3 = psbpool.tile([128, CPAD], FP, space="PSUM", tag="ps3")
                    nmm = int(ch_gqb[g, :, bb].sum())
                    # self-loop diagonal: ps3[slot, :] += h2'[slot]
                    nc.tensor.matmul(out=ps3[:, :], lhsT=ident_t[:, :],
                                     rhs=h2ds[bb][:, :],
                                     start=True, stop=(nmm == 0))
                    done = 0
                    for q in range(Q):
                        b0 = int(ch_gqb[g, q, :bb].sum())
                        for ck in range(int(ch_gqb[g, q, bb])):
                            nc.tensor.matmul(
                                out=ps3[:, :],
                                lhsT=inds[q][:, b0 + ck, :],
                                rhs=gqs[q][:, b0 + ck, :],
                                start=False, stop=(done == nmm - 1))
                            done += 1
                    o3f = midpool.tile([128, CPAD], FP, tag="o3f")
                    nc.scalar.activation(out=o3f[:, :], in_=ps3[:, :],
                                         func=mybir.ActivationFunctionType.Copy,
                                         scale=dp_t[:, blk:blk + 1])
                    o3 = midpool.tile([128, CPAD], FP, tag="o3")
                    nc.vector.tensor_tensor(out=o3[:, :], in0=o3f[:, :],
                                            in1=b2_t[:, :],
                                            op=mybir.AluOpType.add)
                    nc.scalar.dma_start(out=out_s[blk * 128:(blk + 1) * 128, :],
                                        in_=o3[:, :])

    nc.compile()
    return nc


# ======================================================================
# kernel() entry point
# ======================================================================
import os as _os


LAST_EXEC_NS = None
LAST_RES = None


def kernel(x, edge_index, W1, b1, W2, b2):
    """Full-input GCN kernel: shards across 8 NeuronCores internally."""
    global LAST_EXEC_NS, LAST_RES
    import numpy as _np

    trace = bool(int(_os.environ.get("GCN_TRACE", "0")))
    if trace:
        try:
            import sys as _sys
            import types as _types
            from trn_agent_boot.trn_boot import _ntff_profile_via_ctypes
            if "antenv.axon_hooks" not in _sys.modules:
                _hook = _ntff_profile_via_ctypes("/opt/axon/libaxon_pjrt.so")
                _m = _types.ModuleType("antenv.axon_hooks")
                _m.get_axon_ntff_profile_hook = lambda: _hook
                _m.set_axon_ntff_profile_hook = lambda h: None
                _sys.modules["antenv.axon_hooks"] = _m
        except Exception:
            trace = False

    from concourse.bass_utils import run_bass_kernel_spmd

    cfg = Cfg()
    per_core, meta, _ = preprocess(cfg, x, edge_index, W1, b1, W2, b2)
    nc = build(cfg, meta)
    res = run_bass_kernel_spmd(
        nc, per_core, core_ids=list(range(cfg.W)), trace=trace,
    )
    LAST_EXEC_NS = res.exec_time_ns
    LAST_RES = res
    outs = [res.results[c]["out_s"] for c in range(cfg.W)]
    return _np.ascontiguousarray(postprocess(cfg, outs, meta).astype(_np.float32))


# revision 40
# speedup vs baseline: 1.6604x; 1.6604x over previous
"""GCN 2-layer kernel for trn2: host preprocessing + Bass kernel builder.

Math (per GCNConv, PyG-style):
  out = D^-1/2 (A+I) D^-1/2 (X W) + b
Layer1 -> relu -> Layer2.

Device plan (8 cores, SPMD), v2:
  P1: h1' = dinv .* (x_shard @ W1)             (ID-sharded)
  AG1: allgather h1' -> table [NP, HID] bf16   (row r = h1' of node id r)
  P3: per group of G=7 dst blocks: 4 batched dma_gathers (one per src
      quadrant), shared one-hot indicator built once per (group, quadrant),
      per-block indicator matmuls -> psum [HID, 128] (transposed), dinv_dst
      scale (DVE), relu+b1 (ACT), @W2 -> [128, 48], dinv scale -> h2'
  AG2: allgather h2' (128-col padded rows, 48 used) -> [NP, 128] bf16
  P5: per group: 4 batched 96B-payload gathers (48 cols from 256B-pitch
      rows), indicator matmuls -> psum [128, 48], dinv scale + b2 -> out.
Self-loops are regular edges in BOTH layers. Node->position assignment is
quadrant-aligned (pos//QS == id//QS) so both layers share one edge-chunk
structure (same dl/indicator streams; only gather index values differ).
Host: unpermute rows, slice [:N0, :CLS].
"""

from dataclasses import dataclass

import numpy as np

import concourse.bass as bass
import concourse.mybir as mybir
import concourse.tile as tile
from concourse import bacc

FP = mybir.dt.float32
BF = mybir.dt.bfloat16
IND_BUFS = 32


@dataclass
class Cfg:
    N0: int = 100000     # real nodes
    W: int = 8           # cores
    SHARD: int = 12544   # nodes per core (multiple of 128)
    F: int = 512         # in features
    HID: int = 128
    CLS: int = 40
    CPAD: int = 48       # padded class dim (48*2B = 96B gather payload)
    Q: int = 4           # src quadrants (int16 gather index limit)
    G: int = 7           # dst blocks per gather group

    @property
    def NP(self):
        return self.W * self.SHARD

    @property
    def QS(self):
        return self.NP // self.Q

    @property
    def NB(self):
        return self.SHARD // 128  # blocks per core (98)

    @property
    def NG(self):
        return self.NB // self.G  # groups per core (14)


@dataclass
class Meta:
    kq: np.ndarray = None        # [NG, Q, G] chunks per (group, quadrant, block)
    node_of_pos: np.ndarray = None  # [W, SHARD]


def preprocess(cfg: Cfg, x, edge_index, W1, b1, W2, b2):
    N0, W, SHARD, NP, QS = cfg.N0, cfg.W, cfg.SHARD, cfg.NP, cfg.QS
    NB, Q, G, NG = cfg.NB, cfg.Q, cfg.G, cfg.NG
    x = np.asarray(x, dtype=np.float32)
    edge_index = np.asarray(edge_index)
    W1 = np.asarray(W1, np.float32)
    b1 = np.asarray(b1, np.float32)
    W2 = np.asarray(W2, np.float32)
    b2 = np.asarray(b2, np.float32)

    s = edge_index[0].astype(np.int64)
    d = edge_index[1].astype(np.int64)
    loops = np.arange(N0, dtype=np.int64)
    s_all = np.concatenate([s, loops])
    d_all = np.concatenate([d, loops])

    deg = np.bincount(d_all, minlength=NP).astype(np.float64)
    with np.errstate(divide="ignore"):
        dinv = np.where(deg > 0, 1.0 / np.sqrt(deg), 0.0).astype(np.float32)

    # --- degree-balanced position assignment (global serpentine) ---
    nblk = W * NB
    order = np.argsort(-deg[:NP], kind="stable")
    r = np.arange(NP, dtype=np.int64)
    cyc = r % (2 * nblk)
    blk = np.where(cyc < nblk, cyc, 2 * nblk - 1 - cyc)  # serpentine
    slot_ctr = r // (2 * nblk) * 2 + (cyc >= nblk).astype(np.int64)
    pos_of_node = np.empty(NP, dtype=np.int64)
    pos_of_node[order] = (blk % W) * SHARD + (blk // W) * 128 + slot_ctr
    node_of_pos = np.empty(NP, dtype=np.int64)
    node_of_pos[pos_of_node] = np.arange(NP, dtype=np.int64)

    # --- edge routing (self-loops handled as on-device diagonal) ---
    dst_pos = pos_of_node[d]
    src_pos = pos_of_node[s]
    q_e = src_pos // QS

    core_e = dst_pos // SHARD
    blk_e = (dst_pos % SHARD) // 128
    slot_e = dst_pos % 128
    g_e = blk_e // G
    bbg_e = blk_e % G

    # segment key in (core, g, q, bb) order
    key = ((core_e * NG + g_e) * Q + q_e) * G + bbg_e
    nseg = W * NG * Q * G
    counts = np.bincount(key, minlength=nseg).reshape(W, NG, Q, G)
    # exact-max segment lengths (no per-segment 128 rounding); only each
    # (g,q) call stream rounds up to a chunk multiple
    L = counts.max(axis=0).astype(np.int64)  # [NG, Q, G]
    len_gq = L.sum(axis=2)                   # [NG, Q]
    ch_gq_a = np.ceil(len_gq / 128.0).astype(np.int64)
    pad_gq = ch_gq_a * 128
    base = np.zeros(NG * Q, dtype=np.int64)
    base[1:] = np.cumsum(pad_gq.reshape(-1))[:-1]
    base = base.reshape(NG, Q)
    prefL = np.zeros((NG, Q, G + 1), dtype=np.int64)
    prefL[:, :, 1:] = np.cumsum(L, axis=2)
    EPAD = int(pad_gq.sum())
    CT = EPAD // 128  # total chunks

    # per-seg start offset in the padded stream
    seg_off = (base[:, :, None] + prefL[:, :, :G]).reshape(-1)

    # rank of each edge within its (core,g,q,bb) segment
    order = np.argsort(key, kind="stable")
    key_s = key[order]
    seg_start = np.zeros(nseg + 1, dtype=np.int64)
    seg_start[1:] = np.cumsum(counts.reshape(-1))
    rank = np.arange(len(key_s), dtype=np.int64) - seg_start[key_s]
    tgt = seg_off[key_s % (NG * Q * G)] + rank
    core_s = key_s // (NG * Q * G)

    idx_stream = np.zeros((W, EPAD), dtype=np.int16)
    dl_stream = np.full((W, EPAD), -1.0, dtype=np.float32)
    flat = core_s * EPAD + tgt
    idx_stream.reshape(-1)[flat] = (src_pos - q_e * QS)[order].astype(np.int16)
    # dl carries block-in-group * 128 + slot so boundary chunks stay unambiguous
    dlv = (bbg_e * 128 + slot_e).astype(np.float32)
    dl_stream.reshape(-1)[flat] = dlv[order]

    # wrap idx into gather layout: within each (g,q) call's local stream,
    # local index i -> [i % 16, callcol0*8 + i // 16], tiled x8 to 128 parts.
    # call (g,q) spans segments (g,q,0..G-1): local wrap == global wrap as
    # long as call start offsets are multiples of 16 (they are: x128).
    def wrap_idx(stream):
        # stream [W, EPAD]; global position within call cg0*128.. ; since
        # every call boundary is 128-aligned and wrap is (i%16, i//16), a
        # single global wrap per 16 works iff done call-locally. Call starts
        # are 128-aligned => i_local%16 == i_global%16 and
        # i_local//16 == i_global//16 - call0*8. So one global reshape works.
        a = stream.reshape(W, CT, 8, 16)
        wrapped = a.transpose(0, 3, 1, 2).reshape(W, 16, CT * 8)
        return np.tile(wrapped, (1, 8, 1)).astype(np.int16)

    idx_t = wrap_idx(idx_stream)
    dl_t = dl_stream.reshape(W, CT, 128).transpose(0, 2, 1)  # [W,128,CT]

    import ml_dtypes
    bft = ml_dtypes.bfloat16

    dinv_pos = dinv[node_of_pos.reshape(W, SHARD)]  # [W, SHARD] pos-order

    per_core = []
    for c in range(W):
        # host-gathers x rows into position order (pads -> zero rows)
        nop = node_of_pos[c * SHARD:(c + 1) * SHARD]
        xs = np.zeros((SHARD, cfg.F), np.float32)
        real = nop < N0
        xs[real] = x[nop[real]]
        inp = {
            "xT": np.ascontiguousarray(xs.T).astype(bft),          # [F, SHARD]
            "w1": W1.astype(bft),                                  # [F, HID]
            "b1col": b1.reshape(cfg.HID, 1).copy(),                # [HID, 1]
            "w2p": np.pad(W2, ((0, 0), (0, cfg.CPAD - cfg.CLS))).astype(bft),
            "b2rep": np.broadcast_to(
                np.pad(b2, (0, cfg.CPAD - cfg.CLS)), (128, cfg.CPAD)).copy(),
            "iota": np.broadcast_to(
                np.arange(cfg.G * 128, dtype=np.float32),
                (128, cfg.G * 128)).astype(np.float16).copy(),
            "ident": np.eye(128, dtype=np.float32).astype(bft),
            "idx": idx_t[c],
            "dl": dl_t[c].astype(np.float16),
            # pos-order scale: col bb = dinv at positions of block bb
            "dinv_pc": dinv_pos[c].reshape(cfg.NB, 128).T.copy(),  # [128, NB]
            # pos-order row for dst scaling in [HID, slots] orientation
            "dinv_pr": np.broadcast_to(dinv_pos[c], (128, SHARD)).copy(),
        }
        per_core.append(inp)

    meta = Meta(kq=L, node_of_pos=node_of_pos.reshape(W, SHARD))
    return per_core, meta, dinv


def postprocess(cfg: Cfg, outs, meta: Meta):
    """outs: list of [SHARD, CPAD] per core -> [N0, CLS] in node order."""
    res = np.zeros((cfg.NP, cfg.CPAD), np.float32)
    for c in range(cfg.W):
        res[meta.node_of_pos[c]] = outs[c]
    return res[:cfg.N0, :cfg.CLS]


def dma_gather_narrow(nc, out_ap, in_ap, idxs_ap, num_idxs, elem_size,
                      elem_step, queue_num):
    """dma_gather with elem_size_bytes < 256 (payload narrower than the 256B
    row pitch). Mirrors BassGpSimd.dma_gather's non-transpose DRAM path minus
    the %256 payload assert (ucode only requires the *stride* be a multiple
    of 256B; payload is free)."""
    eng = nc.gpsimd
    assert idxs_ap.dtype == mybir.dt.int16
    esz = elem_size * mybir.dt.size(in_ap.dtype)
    assert esz > 0 and esz % 2 == 0
    stride_bytes = elem_step * mybir.dt.size(in_ap.dtype)
    assert stride_bytes % 256 == 0
    stride_bytes_256 = stride_bytes // 256
    assert stride_bytes_256 < 256
    assert in_ap.ap[0][0] == elem_step
    assert in_ap.ap[-1][1] == out_ap.ap[-1][1] == elem_size
    assert out_ap.ap[0][1] * out_ap.ap[1][1] == num_idxs
    _in_ap = eng.lower_ap_dma(in_ap, for_custom_bir_dma=True)
    _idxs_ap = eng.lower_ap(idxs_ap)
    _out_ap = eng.lower_ap(out_ap)
    return eng.add_instruction(
        mybir.InstDMAGatherAnt(
            name=eng.bass.get_next_instruction_name(),
            ins=[*_in_ap, _idxs_ap, eng.lower_val_access(eng.to_reg(num_idxs))],
            outs=[_out_ap],
            transpose=False,
            num_idxs=num_idxs,
            elem_size=elem_size,
            stride_bytes_256=stride_bytes_256,
            gen_mode=0,
            single_packet=False,
            queue_num=queue_num,
            sbuf_tokens_per_rank=0,
            sbuf_free_dim_per_rank=0,
            sbuf_free_dim_pad_per_rank=0,
            sbuf_byte_offset=0,
        )
    )


def build(cfg: Cfg, meta: Meta):
    W, SHARD, NP, F, HID, CPAD = cfg.W, cfg.SHARD, cfg.NP, cfg.F, cfg.HID, cfg.CPAD
    NB, Q, QS, G, NG = cfg.NB, cfg.Q, cfg.QS, cfg.G, cfg.NG
    L = meta.kq  # [NG, Q, G] exact segment lengths
    len_gq = L.sum(axis=2)
    ch_gq = np.ceil(len_gq / 128.0).astype(np.int64)  # chunks per (g,q)
    ch_g = ch_gq.sum(axis=1)
    CT = int(ch_g.sum())
    CHG = int(ch_g.max())
    CHGQ = int(ch_gq.max())
    g_off = np.zeros(NG + 1, dtype=np.int64)
    g_off[1:] = np.cumsum(ch_g)
    prefL = np.zeros((NG, Q, G + 1), dtype=np.int64)
    prefL[:, :, 1:] = np.cumsum(L, axis=2)
    # per (g,q,bb): chunk range [s_ck, e_ck) within the (g,q) stream
    s_ck = prefL[:, :, :G] // 128
    e_ck = -(-prefL[:, :, 1:] // 128)
    e_ck = np.maximum(e_ck, s_ck)  # L==0 -> empty range
    nck = (e_ck - s_ck) * (L > 0)
    NCKMAX = int(nck.max())
    KT = F // 128
    FP16 = mybir.dt.float16

    nc = bacc.Bacc("TRN2", target_bir_lowering=False, debug=False,
                   num_devices=W, num_swdge_queues=4)

    xT = nc.dram_tensor("xT", [F, SHARD], BF, kind="ExternalInput")
    w1 = nc.dram_tensor("w1", [F, HID], BF, kind="ExternalInput")
    b1col = nc.dram_tensor("b1col", [HID, 1], FP, kind="ExternalInput")
    w2p = nc.dram_tensor("w2p", [HID, CPAD], BF, kind="ExternalInput")
    b2rep = nc.dram_tensor("b2rep", [128, CPAD], FP, kind="ExternalInput")
    iota = nc.dram_tensor("iota", [128, G * 128], FP16, kind="ExternalInput")
    ident = nc.dram_tensor("ident", [128, 128], BF, kind="ExternalInput")
    idx = nc.dram_tensor("idx", [128, CT * 8], mybir.dt.int16, kind="ExternalInput")
    dl = nc.dram_tensor("dl", [128, CT], FP16, kind="ExternalInput")
    dinv_pc = nc.dram_tensor("dinv_pc", [128, NB], FP, kind="ExternalInput")
    dinv_pr = nc.dram_tensor("dinv_pr", [128, SHARD], FP, kind="ExternalInput")
    out_s = nc.dram_tensor("out_s", [SHARD, CPAD], FP, kind="ExternalOutput")

    ag1_in = nc.dram_tensor("ag1_in", [SHARD, HID], BF)
    ag1_out = nc.dram_tensor("ag1_out", [NP, HID], BF, addr_space="Shared")
    # L2 table: 256B-pitch rows, only first CPAD cols used
    ag2_in = nc.dram_tensor("ag2_in", [SHARD, 128], BF)
    ag2_out = nc.dram_tensor("ag2_out", [NP, 128], BF, addr_space="Shared")

    qctr = [0]

    def next_q():
        qctr[0] = (qctr[0] + 1) % 4
        return qctr[0]

    with tile.TileContext(nc) as tc:
        with (
            tc.tile_pool(name="const", bufs=1) as cpool,
            tc.tile_pool(name="p1", bufs=4) as p1pool,
            tc.tile_pool(name="meta1", bufs=2) as mpool,
            tc.tile_pool(name="gath1", bufs=2) as g1pool,
            tc.tile_pool(name="gath2", bufs=2) as g2pool,
            tc.tile_pool(name="indp", bufs=IND_BUFS) as ipool,
            tc.tile_pool(name="mid", bufs=3) as midpool,
            tc.tile_pool(name="psa", bufs=2, space="PSUM") as psapool,
            tc.tile_pool(name="psb", bufs=2, space="PSUM") as psbpool,
        ):
            # ---- constants ----
            iota_t = cpool.tile([128, G * 128], FP16)
            nc.sync.dma_start(out=iota_t[:, :], in_=iota[:, :])
            ident_t = cpool.tile([128, 128], BF)
            nc.sync.dma_start(out=ident_t[:, :], in_=ident[:, :])
            b1_t = cpool.tile([HID, 1], FP)
            nc.sync.dma_start(out=b1_t[:, :], in_=b1col[:, :])
            w2_t = cpool.tile([HID, CPAD], BF)
            nc.sync.dma_start(out=w2_t[:, :], in_=w2p[:, :])
            b2_t = cpool.tile([128, CPAD], FP)
            nc.sync.dma_start(out=b2_t[:, :], in_=b2rep[:, :])
            dp_t = cpool.tile([128, NB], FP)
            nc.sync.dma_start(out=dp_t[:, :], in_=dinv_pc[:, :])
            w1k_t = cpool.tile([128, KT, HID], BF)
            for k in range(KT):
                nc.sync.dma_start(out=w1k_t[:, k, :], in_=w1[k * 128:(k + 1) * 128, :])

            # ---- phase 1: h1' = dinv .* (x @ W1) ----
            for t in range(NB):
                psh = psapool.tile([128, HID], FP, space="PSUM", tag="ph1")
                for k in range(KT):
                    xt_t = p1pool.tile([128, 128], BF, tag="xt")
                    nc.sync.dma_start(
                        out=xt_t[:, :],
                        in_=xT[k * 128:(k + 1) * 128, t * 128:(t + 1) * 128])
                    nc.tensor.matmul(out=psh[:, :], lhsT=xt_t[:, :],
                                     rhs=w1k_t[:, k, :],
                                     start=(k == 0), stop=(k == KT - 1))
                h1p = p1pool.tile([128, HID], BF, tag="h1p")
                nc.scalar.activation(out=h1p[:, :], in_=psh[:, :],
                                     func=mybir.ActivationFunctionType.Copy,
                                     scale=dp_t[:, t:t + 1])
                nc.scalar.dma_start(out=ag1_in[t * 128:(t + 1) * 128, :],
                                    in_=h1p[:, :])

            # ---- AG1 ----
            nc.gpsimd.collective_compute(
                "AllGather", mybir.AluOpType.bypass,
                replica_groups=[list(range(W))],
                ins=[ag1_in[:, :]], outs=[ag1_out[:, :]],
            )

            # ================= phase 3: L1 agg + relu + @W2 =================
            def load_group_meta(g, idx_dram):
                c0, c1 = int(g_off[g]), int(g_off[g + 1])
                ncch = c1 - c0
                ixt = mpool.tile([128, CHG * 8], mybir.dt.int16, tag="ix",
                                 bufs=3)
                nc.sync.dma_start(out=ixt[:, :ncch * 8],
                                  in_=idx_dram[:, c0 * 8:c1 * 8])
                dlt = mpool.tile([128, CHG], FP16, tag="dl", bufs=3)
                nc.sync.dma_start(out=dlt[:, :ncch], in_=dl[:, c0:c1])
                return ixt, dlt

            def build_inds(g, dlt):
                """one indicator tile per (g, q, bb) over its chunk range;
                dl carries bbg*128+slot so rows of other blocks compare to 0"""
                inds = {}
                co = 0
                for q in range(Q):
                    for bb in range(G):
                        n = int(nck[g, q, bb])
                        if n == 0:
                            inds[(q, bb)] = None
                            continue
                        s0 = int(s_ck[g, q, bb])
                        ind = ipool.tile([128, NCKMAX, 128], BF, tag="ind")
                        nc.vector.tensor_tensor(
                            out=ind[:, :n, :],
                            in0=dlt[:, co + s0:co + s0 + n]
                                .to_broadcast([128, n, 128]),
                            in1=iota_t[:, None, bb * 128:(bb + 1) * 128]
                                .to_broadcast([128, n, 128]),
                            op=mybir.AluOpType.is_equal,
                        )
                        inds[(q, bb)] = ind
                    co += int(ch_gq[g, q])
                return inds

            # calls capped at CAPCH chunks (2048 idxs = 128 descs/engine) so a
            # whole call fits the SDMA descriptor ring without Q7 busy-waiting
            CAPCH = 8

            def gather_group(g, pool, ixt, table, width, narrow, tag):
                """q-round-robin sub-calls so all quadrants complete evenly"""
                tiles = [pool.tile([128, CHGQ, width], BF, tag=tag, bufs=8,
                                   name=f"gq{tag}{g}_{q}")
                         for q in range(Q)]
                s = 0
                while True:
                    any_left = False
                    for q in range(Q):
                        co = int(ch_gq[g, :q].sum())
                        nq = int(ch_gq[g, q])
                        if s >= nq:
                            continue
                        any_left = True
                        n = min(CAPCH, nq - s)
                        if narrow:
                            dma_gather_narrow(
                                nc, tiles[q][:, s:s + n, :],
                                table[q * QS:(q + 1) * QS, :width],
                                ixt[:, (co + s) * 8:(co + s + n) * 8],
                                n * 128, width, 128, next_q())
                        else:
                            nc.gpsimd.dma_gather(
                                tiles[q][:, s:s + n, :],
                                table[q * QS:(q + 1) * QS, :],
                                ixt[:, (co + s) * 8:(co + s + n) * 8],
                                n * 128, n * 128, width,
                                single_packet=False, queue_num=next_q(),
                            )
                    if not any_left:
                        break
                    s += CAPCH
                return tiles

            def load_drt(g):
                drt = mpool.tile([128, G * 128], FP, tag="drt", bufs=3)
                nc.sync.dma_start(
                    out=drt[:, :],
                    in_=dinv_pr[:, g * G * 128:(g + 1) * G * 128])
                return drt

            def load_h1ds(g):
                hs = []
                for bb in range(G):
                    blk = g * G + bb
                    h1d = mpool.tile([128, HID], BF, tag="h1d", bufs=16)
                    nc.sync.dma_start(
                        out=h1d[:, :],
                        in_=ag1_in[blk * 128:(blk + 1) * 128, :])
                    hs.append(h1d)
                return hs

            # prologue: two-group lookahead of metadata (flows during AG1)
            p3_meta = {0: load_group_meta(0, idx), 1: load_group_meta(1, idx)}
            p3_inds = {0: build_inds(0, p3_meta[0][1])}
            p3_drt = {0: load_drt(0)}
            p3_h1d = {0: load_h1ds(0)}

            for g in range(NG):
                gqs = gather_group(g, g1pool, p3_meta[g][0], ag1_out, HID,
                                   False, "g1")
                # prefetch: meta two ahead, inds/drt/diag one ahead
                if g + 2 < NG:
                    p3_meta[g + 2] = load_group_meta(g + 2, idx)
                if g + 1 < NG:
                    p3_inds[g + 1] = build_inds(g + 1, p3_meta[g + 1][1])
                    p3_drt[g + 1] = load_drt(g + 1)
                    p3_h1d[g + 1] = load_h1ds(g + 1)

                inds = p3_inds[g]
                drt = p3_drt[g]
                h1ds = p3_h1d[g]
                for bb in range(G):
                    blk = g * G + bb
                    ps1 = psapool.tile([128, 128], FP, space="PSUM", tag="ps1")
                    nmm = int(nck[g, :, bb].sum())
                    # self-loop diagonal: ps1[:, j] += h1'[slot j] (transpose
                    # via identity matmul; dst dinv applied later via drt)
                    nc.tensor.matmul(out=ps1[:, :], lhsT=h1ds[bb][:, :],
                                     rhs=ident_t[:, :],
                                     start=True, stop=(nmm == 0))
                    done = 0
                    for q in range(Q):
                        s0 = int(s_ck[g, q, bb])
                        for ck in range(int(nck[g, q, bb])):
                            nc.tensor.matmul(
                                out=ps1[:, :],
                                lhsT=gqs[q][:, s0 + ck, :],
                                rhs=inds[(q, bb)][:, ck, :],
                                start=False, stop=(done == nmm - 1))
                            done += 1
                    t1 = midpool.tile([128, 128], FP, tag="t1")
                    nc.vector.tensor_tensor(
                        out=t1[:, :], in0=ps1[:, :],
                        in1=drt[:, bb * 128:(bb + 1) * 128],
                        op=mybir.AluOpType.mult)
                    r1 = midpool.tile([128, 128], BF, tag="r1")
                    nc.scalar.activation(out=r1[:, :], in_=t1[:, :],
                                         func=mybir.ActivationFunctionType.Relu,
                                         bias=b1_t[:, :1])
                    ps2 = psbpool.tile([128, CPAD], FP, space="PSUM", tag="ps2")
                    nc.tensor.matmul(out=ps2[:, :], lhsT=r1[:, :], rhs=w2_t[:, :],
                                     start=True, stop=True)
                    h2p = midpool.tile([128, CPAD], BF, tag="h2p")
                    nc.scalar.activation(out=h2p[:, :], in_=ps2[:, :],
                                         func=mybir.ActivationFunctionType.Copy,
                                         scale=dp_t[:, blk:blk + 1])
                    nc.scalar.dma_start(
                        out=ag2_in[blk * 128:(blk + 1) * 128, :CPAD],
                        in_=h2p[:, :])

            # ---- AG2 ----
            nc.gpsimd.collective_compute(
                "AllGather", mybir.AluOpType.bypass,
                replica_groups=[list(range(W))],
                ins=[ag2_in[:, :]], outs=[ag2_out[:, :]],
            )

            # ================= phase 5: L2 agg + b2 -> out =================
            def load_h2ds(g):
                hs = []
                for bb in range(G):
                    blk = g * G + bb
                    h2d = mpool.tile([128, CPAD], BF, tag="h2d", bufs=16)
                    nc.sync.dma_start(
                        out=h2d[:, :],
                        in_=ag2_in[blk * 128:(blk + 1) * 128, :CPAD])
                    hs.append(h2d)
                return hs

            p5_meta = {0: load_group_meta(0, idx), 1: load_group_meta(1, idx)}
            p5_inds = {0: build_inds(0, p5_meta[0][1])}
            p5_h2d = {0: load_h2ds(0)}

            for g in range(NG):
                gqs = gather_group(g, g2pool, p5_meta[g][0], ag2_out, CPAD,
                                   True, "g2")
                if g + 2 < NG:
                    p5_meta[g + 2] = load_group_meta(g + 2, idx)
                if g + 1 < NG:
                    p5_inds[g + 1] = build_inds(g + 1, p5_meta[g + 1][1])
                    p5_h2d[g + 1] = load_h2ds(g + 1)

                inds = p5_inds[g]
                h2ds = p5_h2d[g]
                for bb in range(G):
                    blk = g * G + bb
                    ps3 = psbpool.tile([128, CPAD], FP, space="PSUM", tag="ps3")
                    nmm = int(nck[g, :, bb].sum())
                    # self-loop diagonal: ps3[slot, :] += h2'[slot]
                    nc.tensor.matmul(out=ps3[:, :], lhsT=ident_t[:, :],
                                     rhs=h2ds[bb][:, :],
                                     start=True, stop=(nmm == 0))
                    done = 0
                    for q in range(Q):
                        s0 = int(s_ck[g, q, bb])
                        for ck in range(int(nck[g, q, bb])):
                            nc.tensor.matmul(
                                out=ps3[:, :],
                                lhsT=inds[(q, bb)][:, ck, :],
                                rhs=gqs[q][:, s0 + ck, :],
                                start=False, stop=(done == nmm - 1))
                            done += 1
                    o3f = midpool.tile([128, CPAD], FP, tag="o3f")
                    nc.scalar.activation(out=o3f[:, :], in_=ps3[:, :],
                                         func=mybir.ActivationFunctionType.Copy,
                                         scale=dp_t[:, blk:blk + 1])
                    o3 = midpool.tile([128, CPAD], FP, tag="o3")
                    nc.vector.tensor_tensor(out=o3[:, :], in0=o3f[:, :],
                                            in1=b2_t[:, :],
                                            op=mybir.AluOpType.add)
                    nc.scalar.dma_start(out=out_s[blk * 128:(blk + 1) * 128, :],
                                        in_=o3[:, :])

    nc.compile()
    return nc


# ======================================================================
# kernel() entry point
# ======================================================================
import os as _os


LAST_EXEC_NS = None
LAST_RES = None


def kernel(x, edge_index, W1, b1, W2, b2):
    """Full-input GCN kernel: shards across 8 NeuronCores internally."""
    global LAST_EXEC_NS, LAST_RES
    import numpy as _np

    trace = bool(int(_os.environ.get("GCN_TRACE", "0")))
    if trace:
        try:
            import sys as _sys
            import types as _types
            from trn_agent_boot.trn_boot import _ntff_profile_via_ctypes
            if "antenv.axon_hooks" not in _sys.modules:
                _hook = _ntff_profile_via_ctypes("/opt/axon/libaxon_pjrt.so")
                _m = _types.ModuleType("antenv.axon_hooks")
                _m.get_axon_ntff_profile_hook = lambda: _hook
                _m.set_axon_ntff_profile_hook = lambda h: None
                _sys.modules["antenv.axon_hooks"] = _m
        except Exception:
            trace = False

    from concourse.bass_utils import run_bass_kernel_spmd

    cfg = Cfg()
    per_core, meta, _ = preprocess(cfg, x, edge_index, W1, b1, W2, b2)
    nc = build(cfg, meta)
    res = run_bass_kernel_spmd(
        nc, per_core, core_ids=list(range(cfg.W)), trace=trace,
    )
    LAST_EXEC_NS = res.exec_time_ns
    LAST_RES = res
    outs = [res.results[c]["out_s"] for c in range(cfg.W)]
    return _np.ascontiguousarray(postprocess(cfg, outs, meta).astype(_np.float32))


# revision 42
# speedup vs baseline: 1.8928x; 1.1400x over previous
"""GCN 2-layer kernel for trn2: host preprocessing + Bass kernel builder.

Math (per GCNConv, PyG-style):
  out = D^-1/2 (A+I) D^-1/2 (X W) + b
Layer1 -> relu -> Layer2.

Device plan (8 cores, SPMD):
  P1: h1' = dinv .* (x_pos @ W1)   (position-ordered; x host-permuted)
  AG1: allgather h1' -> table [NP, HID] bf16 (row = position)
  P3: per group of G=7 dst blocks: quadrant-round-robin batched dma_gathers
      (<=1024 idxs/call to fit the SDMA ring), per-(group,quadrant,block)
      one-hot indicators (dl = blockInGroup*128+slot vs iota, fp16),
      per-block indicator matmuls -> psum [HID, 128] (transposed), self-loop
      diagonal via identity matmul of the local row block, dinv_dst scale
      (DVE), relu+b1 (ACT), @W2 -> [128, 48], dinv scale -> h2'
  AG2: allgather h2' (256B-pitch rows, first 48 cols used) -> [NP, 128] bf16
  P5: same structure with 96B-payload narrow gathers + b2.
Edge streams use exact-max (cross-core) segment padding; only each
(group, quadrant) call stream rounds up to a 128 multiple.
Host: unpermute rows, slice [:N0, :CLS].
"""

from dataclasses import dataclass

import numpy as np

import concourse.bass as bass
import concourse.mybir as mybir
import concourse.tile as tile
from concourse import bacc

FP = mybir.dt.float32
BF = mybir.dt.bfloat16
IND_BUFS = 32


@dataclass
class Cfg:
    N0: int = 100000     # real nodes
    W: int = 8           # cores
    SHARD: int = 12544   # nodes per core (multiple of 128)
    F: int = 512         # in features
    HID: int = 128
    CLS: int = 40
    CPAD: int = 48       # padded class dim (48*2B = 96B gather payload)
    Q: int = 4           # src quadrants (int16 gather index limit)
    G: int = 7           # dst blocks per gather group

    @property
    def NP(self):
        return self.W * self.SHARD

    @property
    def QS(self):
        return self.NP // self.Q

    @property
    def NB(self):
        return self.SHARD // 128  # blocks per core (98)

    @property
    def NG(self):
        return self.NB // self.G  # groups per core (14)


@dataclass
class Meta:
    kq: np.ndarray = None        # [NG, Q, G] chunks per (group, quadrant, block)
    node_of_pos: np.ndarray = None  # [W, SHARD]


def preprocess(cfg: Cfg, x, edge_index, W1, b1, W2, b2):
    N0, W, SHARD, NP, QS = cfg.N0, cfg.W, cfg.SHARD, cfg.NP, cfg.QS
    NB, Q, G, NG = cfg.NB, cfg.Q, cfg.G, cfg.NG
    x = np.asarray(x, dtype=np.float32)
    edge_index = np.asarray(edge_index)
    W1 = np.asarray(W1, np.float32)
    b1 = np.asarray(b1, np.float32)
    W2 = np.asarray(W2, np.float32)
    b2 = np.asarray(b2, np.float32)

    s = edge_index[0].astype(np.int64)
    d = edge_index[1].astype(np.int64)
    loops = np.arange(N0, dtype=np.int64)
    s_all = np.concatenate([s, loops])
    d_all = np.concatenate([d, loops])

    deg = np.bincount(d_all, minlength=NP).astype(np.float64)
    with np.errstate(divide="ignore"):
        dinv = np.where(deg > 0, 1.0 / np.sqrt(deg), 0.0).astype(np.float32)

    # --- degree-balanced position assignment (global serpentine) ---
    nblk = W * NB
    order = np.argsort(-deg[:NP], kind="stable")
    r = np.arange(NP, dtype=np.int64)
    cyc = r % (2 * nblk)
    blk = np.where(cyc < nblk, cyc, 2 * nblk - 1 - cyc)  # serpentine
    slot_ctr = r // (2 * nblk) * 2 + (cyc >= nblk).astype(np.int64)
    pos_of_node = np.empty(NP, dtype=np.int64)
    pos_of_node[order] = (blk % W) * SHARD + (blk // W) * 128 + slot_ctr
    node_of_pos = np.empty(NP, dtype=np.int64)
    node_of_pos[pos_of_node] = np.arange(NP, dtype=np.int64)

    # --- edge routing (self-loops handled as on-device diagonal) ---
    dst_pos = pos_of_node[d]
    src_pos = pos_of_node[s]
    q_e = src_pos // QS

    core_e = dst_pos // SHARD
    blk_e = (dst_pos % SHARD) // 128
    slot_e = dst_pos % 128
    g_e = blk_e // G
    bbg_e = blk_e % G

    # segment key in (core, g, q, bb) order
    key = ((core_e * NG + g_e) * Q + q_e) * G + bbg_e
    nseg = W * NG * Q * G
    counts = np.bincount(key, minlength=nseg).reshape(W, NG, Q, G)
    # exact-max segment lengths (no per-segment 128 rounding); only each
    # (g,q) call stream rounds up to a chunk multiple
    L = counts.max(axis=0).astype(np.int64)  # [NG, Q, G]
    len_gq = L.sum(axis=2)                   # [NG, Q]
    ch_gq_a = np.ceil(len_gq / 128.0).astype(np.int64)
    pad_gq = ch_gq_a * 128
    base = np.zeros(NG * Q, dtype=np.int64)
    base[1:] = np.cumsum(pad_gq.reshape(-1))[:-1]
    base = base.reshape(NG, Q)
    prefL = np.zeros((NG, Q, G + 1), dtype=np.int64)
    prefL[:, :, 1:] = np.cumsum(L, axis=2)
    EPAD = int(pad_gq.sum())
    CT = EPAD // 128  # total chunks

    # per-seg start offset in the padded stream
    seg_off = (base[:, :, None] + prefL[:, :, :G]).reshape(-1)

    # rank of each edge within its (core,g,q,bb) segment
    order = np.argsort(key, kind="stable")
    key_s = key[order]
    seg_start = np.zeros(nseg + 1, dtype=np.int64)
    seg_start[1:] = np.cumsum(counts.reshape(-1))
    rank = np.arange(len(key_s), dtype=np.int64) - seg_start[key_s]
    tgt = seg_off[key_s % (NG * Q * G)] + rank
    core_s = key_s // (NG * Q * G)

    idx_stream = np.zeros((W, EPAD), dtype=np.int16)
    dl_stream = np.full((W, EPAD), -1.0, dtype=np.float32)
    flat = core_s * EPAD + tgt
    idx_stream.reshape(-1)[flat] = (src_pos - q_e * QS)[order].astype(np.int16)
    # dl carries block-in-group * 128 + slot so boundary chunks stay unambiguous
    dlv = (bbg_e * 128 + slot_e).astype(np.float32)
    dl_stream.reshape(-1)[flat] = dlv[order]

    # wrap idx into gather layout: within each (g,q) call's local stream,
    # local index i -> [i % 16, callcol0*8 + i // 16], tiled x8 to 128 parts.
    # call (g,q) spans segments (g,q,0..G-1): local wrap == global wrap as
    # long as call start offsets are multiples of 16 (they are: x128).
    def wrap_idx(stream):
        # stream [W, EPAD]; global position within call cg0*128.. ; since
        # every call boundary is 128-aligned and wrap is (i%16, i//16), a
        # single global wrap per 16 works iff done call-locally. Call starts
        # are 128-aligned => i_local%16 == i_global%16 and
        # i_local//16 == i_global//16 - call0*8. So one global reshape works.
        a = stream.reshape(W, CT, 8, 16)
        wrapped = a.transpose(0, 3, 1, 2).reshape(W, 16, CT * 8)
        return np.tile(wrapped, (1, 8, 1)).astype(np.int16)

    idx_t = wrap_idx(idx_stream)
    dl_t = dl_stream.reshape(W, CT, 128).transpose(0, 2, 1)  # [W,128,CT]

    import ml_dtypes
    bft = ml_dtypes.bfloat16

    dinv_pos = dinv[node_of_pos.reshape(W, SHARD)]  # [W, SHARD] pos-order

    per_core = []
    for c in range(W):
        # host-gathers x rows into position order (pads -> zero rows)
        nop = node_of_pos[c * SHARD:(c + 1) * SHARD]
        xs = np.zeros((SHARD, cfg.F), np.float32)
        real = nop < N0
        xs[real] = x[nop[real]]
        inp = {
            "xT": np.ascontiguousarray(xs.T).astype(bft),          # [F, SHARD]
            "w1": W1.astype(bft),                                  # [F, HID]
            "b1col": b1.reshape(cfg.HID, 1).copy(),                # [HID, 1]
            "w2p": np.pad(W2, ((0, 0), (0, cfg.CPAD - cfg.CLS))).astype(bft),
            "b2rep": np.broadcast_to(
                np.pad(b2, (0, cfg.CPAD - cfg.CLS)), (128, cfg.CPAD)).copy(),
            "iota": np.broadcast_to(
                np.arange(cfg.G * 128, dtype=np.float32),
                (128, cfg.G * 128)).astype(np.float16).copy(),
            "ident": np.eye(128, dtype=np.float32).astype(bft),
            "idx": idx_t[c],
            "dl": dl_t[c].astype(np.float16),
            # pos-order scale: col bb = dinv at positions of block bb
            "dinv_pc": dinv_pos[c].reshape(cfg.NB, 128).T.copy(),  # [128, NB]
            # pos-order row for dst scaling in [HID, slots] orientation
            "dinv_pr": np.broadcast_to(dinv_pos[c], (128, SHARD)).copy(),
        }
        per_core.append(inp)

    meta = Meta(kq=L, node_of_pos=node_of_pos.reshape(W, SHARD))
    return per_core, meta, dinv


def postprocess(cfg: Cfg, outs, meta: Meta):
    """outs: list of [SHARD, CPAD] per core -> [N0, CLS] in node order."""
    res = np.zeros((cfg.NP, cfg.CPAD), np.float32)
    for c in range(cfg.W):
        res[meta.node_of_pos[c]] = outs[c]
    return res[:cfg.N0, :cfg.CLS]


def dma_gather_narrow(nc, out_ap, in_ap, idxs_ap, num_idxs, elem_size,
                      elem_step, queue_num):
    """dma_gather with elem_size_bytes < 256 (payload narrower than the 256B
    row pitch). Mirrors BassGpSimd.dma_gather's non-transpose DRAM path minus
    the %256 payload assert (ucode only requires the *stride* be a multiple
    of 256B; payload is free)."""
    eng = nc.gpsimd
    assert idxs_ap.dtype == mybir.dt.int16
    esz = elem_size * mybir.dt.size(in_ap.dtype)
    assert esz > 0 and esz % 2 == 0
    stride_bytes = elem_step * mybir.dt.size(in_ap.dtype)
    assert stride_bytes % 256 == 0
    stride_bytes_256 = stride_bytes // 256
    assert stride_bytes_256 < 256
    assert in_ap.ap[0][0] == elem_step
    assert in_ap.ap[-1][1] == out_ap.ap[-1][1] == elem_size
    assert out_ap.ap[0][1] * out_ap.ap[1][1] == num_idxs
    _in_ap = eng.lower_ap_dma(in_ap, for_custom_bir_dma=True)
    _idxs_ap = eng.lower_ap(idxs_ap)
    _out_ap = eng.lower_ap(out_ap)
    return eng.add_instruction(
        mybir.InstDMAGatherAnt(
            name=eng.bass.get_next_instruction_name(),
            ins=[*_in_ap, _idxs_ap, eng.lower_val_access(eng.to_reg(num_idxs))],
            outs=[_out_ap],
            transpose=False,
            num_idxs=num_idxs,
            elem_size=elem_size,
            stride_bytes_256=stride_bytes_256,
            gen_mode=0,
            single_packet=False,
            queue_num=queue_num,
            sbuf_tokens_per_rank=0,
            sbuf_free_dim_per_rank=0,
            sbuf_free_dim_pad_per_rank=0,
            sbuf_byte_offset=0,
        )
    )


def build(cfg: Cfg, meta: Meta):
    W, SHARD, NP, F, HID, CPAD = cfg.W, cfg.SHARD, cfg.NP, cfg.F, cfg.HID, cfg.CPAD
    NB, Q, QS, G, NG = cfg.NB, cfg.Q, cfg.QS, cfg.G, cfg.NG
    L = meta.kq  # [NG, Q, G] exact segment lengths
    len_gq = L.sum(axis=2)
    ch_gq = np.ceil(len_gq / 128.0).astype(np.int64)  # chunks per (g,q)
    ch_g = ch_gq.sum(axis=1)
    CT = int(ch_g.sum())
    CHG = int(ch_g.max())
    CHGQ = int(ch_gq.max())
    g_off = np.zeros(NG + 1, dtype=np.int64)
    g_off[1:] = np.cumsum(ch_g)
    prefL = np.zeros((NG, Q, G + 1), dtype=np.int64)
    prefL[:, :, 1:] = np.cumsum(L, axis=2)
    # per (g,q,bb): chunk range [s_ck, e_ck) within the (g,q) stream
    s_ck = prefL[:, :, :G] // 128
    e_ck = -(-prefL[:, :, 1:] // 128)
    e_ck = np.maximum(e_ck, s_ck)  # L==0 -> empty range
    nck = (e_ck - s_ck) * (L > 0)
    NCKMAX = int(nck.max())
    KT = F // 128
    FP16 = mybir.dt.float16

    nc = bacc.Bacc("TRN2", target_bir_lowering=False, debug=False,
                   num_devices=W, num_swdge_queues=4)

    xT = nc.dram_tensor("xT", [F, SHARD], BF, kind="ExternalInput")
    w1 = nc.dram_tensor("w1", [F, HID], BF, kind="ExternalInput")
    b1col = nc.dram_tensor("b1col", [HID, 1], FP, kind="ExternalInput")
    w2p = nc.dram_tensor("w2p", [HID, CPAD], BF, kind="ExternalInput")
    b2rep = nc.dram_tensor("b2rep", [128, CPAD], FP, kind="ExternalInput")
    iota = nc.dram_tensor("iota", [128, G * 128], FP16, kind="ExternalInput")
    ident = nc.dram_tensor("ident", [128, 128], BF, kind="ExternalInput")
    idx = nc.dram_tensor("idx", [128, CT * 8], mybir.dt.int16, kind="ExternalInput")
    dl = nc.dram_tensor("dl", [128, CT], FP16, kind="ExternalInput")
    dinv_pc = nc.dram_tensor("dinv_pc", [128, NB], FP, kind="ExternalInput")
    dinv_pr = nc.dram_tensor("dinv_pr", [128, SHARD], FP, kind="ExternalInput")
    out_s = nc.dram_tensor("out_s", [SHARD, CPAD], FP, kind="ExternalOutput")

    ag1_in = nc.dram_tensor("ag1_in", [SHARD, HID], BF)
    ag1_out = nc.dram_tensor("ag1_out", [NP, HID], BF, addr_space="Shared")
    # L2 table: 256B-pitch rows, only first CPAD cols used
    ag2_in = nc.dram_tensor("ag2_in", [SHARD, 128], BF)
    ag2_out = nc.dram_tensor("ag2_out", [NP, 128], BF, addr_space="Shared")

    qctr = [0]

    def next_q():
        qctr[0] = (qctr[0] + 1) % 4
        return qctr[0]

    with tile.TileContext(nc) as tc:
        with (
            tc.tile_pool(name="const", bufs=1) as cpool,
            tc.tile_pool(name="p1", bufs=4) as p1pool,
            tc.tile_pool(name="meta1", bufs=2) as mpool,
            tc.tile_pool(name="gath1", bufs=2) as g1pool,
            tc.tile_pool(name="gath2", bufs=2) as g2pool,
            tc.tile_pool(name="indp", bufs=IND_BUFS) as ipool,
            tc.tile_pool(name="mid", bufs=3) as midpool,
            tc.tile_pool(name="psa", bufs=2, space="PSUM") as psapool,
            tc.tile_pool(name="psb", bufs=2, space="PSUM") as psbpool,
        ):
            # ---- constants ----
            iota_t = cpool.tile([128, G * 128], FP16)
            nc.sync.dma_start(out=iota_t[:, :], in_=iota[:, :])
            ident_t = cpool.tile([128, 128], BF)
            nc.sync.dma_start(out=ident_t[:, :], in_=ident[:, :])
            b1_t = cpool.tile([HID, 1], FP)
            nc.sync.dma_start(out=b1_t[:, :], in_=b1col[:, :])
            w2_t = cpool.tile([HID, CPAD], BF)
            nc.sync.dma_start(out=w2_t[:, :], in_=w2p[:, :])
            b2_t = cpool.tile([128, CPAD], FP)
            nc.sync.dma_start(out=b2_t[:, :], in_=b2rep[:, :])
            dp_t = cpool.tile([128, NB], FP)
            nc.sync.dma_start(out=dp_t[:, :], in_=dinv_pc[:, :])
            w1k_t = cpool.tile([128, KT, HID], BF)
            for k in range(KT):
                nc.sync.dma_start(out=w1k_t[:, k, :], in_=w1[k * 128:(k + 1) * 128, :])

            # ---- phase 1: h1' = dinv .* (x @ W1) ----
            # load x in 7-block column chunks (few big DMAs, not 392 small)
            TC = 7 * 128
            for ch in range(NB // 7):
                xc = p1pool.tile([128, KT, TC], BF, tag="xc", bufs=2)
                for k in range(KT):
                    nc.sync.dma_start(
                        out=xc[:, k, :],
                        in_=xT[k * 128:(k + 1) * 128, ch * TC:(ch + 1) * TC])
                for tt in range(7):
                    t = ch * 7 + tt
                    psh = psapool.tile([128, HID], FP, space="PSUM", tag="ph1")
                    for k in range(KT):
                        nc.tensor.matmul(
                            out=psh[:, :],
                            lhsT=xc[:, k, tt * 128:(tt + 1) * 128],
                            rhs=w1k_t[:, k, :],
                            start=(k == 0), stop=(k == KT - 1))
                    h1p = p1pool.tile([128, HID], BF, tag="h1p")
                    nc.scalar.activation(out=h1p[:, :], in_=psh[:, :],
                                         func=mybir.ActivationFunctionType.Copy,
                                         scale=dp_t[:, t:t + 1])
                    nc.scalar.dma_start(out=ag1_in[t * 128:(t + 1) * 128, :],
                                        in_=h1p[:, :])

            # ---- AG1 ----
            nc.gpsimd.collective_compute(
                "AllGather", mybir.AluOpType.bypass,
                replica_groups=[list(range(W))],
                ins=[ag1_in[:, :]], outs=[ag1_out[:, :]],
            )

            # ================= phase 3: L1 agg + relu + @W2 =================
            def load_group_meta(g, idx_dram):
                c0, c1 = int(g_off[g]), int(g_off[g + 1])
                ncch = c1 - c0
                ixt = mpool.tile([128, CHG * 8], mybir.dt.int16, tag="ix",
                                 bufs=3)
                nc.sync.dma_start(out=ixt[:, :ncch * 8],
                                  in_=idx_dram[:, c0 * 8:c1 * 8])
                dlt = mpool.tile([128, CHG], FP16, tag="dl", bufs=3)
                nc.sync.dma_start(out=dlt[:, :ncch], in_=dl[:, c0:c1])
                return ixt, dlt

            def build_inds(g, dlt):
                """one indicator tile per (g, q, bb) over its chunk range;
                dl carries bbg*128+slot so rows of other blocks compare to 0"""
                inds = {}
                co = 0
                for q in range(Q):
                    for bb in range(G):
                        n = int(nck[g, q, bb])
                        if n == 0:
                            inds[(q, bb)] = None
                            continue
                        s0 = int(s_ck[g, q, bb])
                        ind = ipool.tile([128, NCKMAX, 128], BF, tag="ind")
                        nc.vector.tensor_tensor(
                            out=ind[:, :n, :],
                            in0=dlt[:, co + s0:co + s0 + n]
                                .to_broadcast([128, n, 128]),
                            in1=iota_t[:, None, bb * 128:(bb + 1) * 128]
                                .to_broadcast([128, n, 128]),
                            op=mybir.AluOpType.is_equal,
                        )
                        inds[(q, bb)] = ind
                    co += int(ch_gq[g, q])
                return inds

            # calls capped at CAPCH chunks (2048 idxs = 128 descs/engine) so a
            # whole call fits the SDMA descriptor ring without Q7 busy-waiting
            CAPCH = 8

            def gather_group(g, pool, ixt, table, width, narrow, tag):
                """q-round-robin sub-calls so all quadrants complete evenly"""
                tiles = [pool.tile([128, CHGQ, width], BF, tag=tag, bufs=8,
                                   name=f"gq{tag}{g}_{q}")
                         for q in range(Q)]
                s = 0
                while True:
                    any_left = False
                    for q in range(Q):
                        co = int(ch_gq[g, :q].sum())
                        nq = int(ch_gq[g, q])
                        if s >= nq:
                            continue
                        any_left = True
                        n = min(CAPCH, nq - s)
                        if narrow:
                            dma_gather_narrow(
                                nc, tiles[q][:, s:s + n, :],
                                table[q * QS:(q + 1) * QS, :width],
                                ixt[:, (co + s) * 8:(co + s + n) * 8],
                                n * 128, width, 128, next_q())
                        else:
                            nc.gpsimd.dma_gather(
                                tiles[q][:, s:s + n, :],
                                table[q * QS:(q + 1) * QS, :],
                                ixt[:, (co + s) * 8:(co + s + n) * 8],
                                n * 128, n * 128, width,
                                single_packet=False, queue_num=next_q(),
                            )
                    if not any_left:
                        break
                    s += CAPCH
                return tiles

            def load_drt(g):
                drt = mpool.tile([128, G * 128], FP, tag="drt", bufs=3)
                nc.sync.dma_start(
                    out=drt[:, :],
                    in_=dinv_pr[:, g * G * 128:(g + 1) * G * 128])
                return drt

            def load_h1ds(g):
                hs = []
                for bb in range(G):
                    blk = g * G + bb
                    h1d = mpool.tile([128, HID], BF, tag="h1d", bufs=16)
                    nc.sync.dma_start(
                        out=h1d[:, :],
                        in_=ag1_in[blk * 128:(blk + 1) * 128, :])
                    hs.append(h1d)
                return hs

            # prologue: two-group lookahead of metadata (flows during AG1)
            p3_meta = {0: load_group_meta(0, idx), 1: load_group_meta(1, idx)}
            p3_inds = {0: build_inds(0, p3_meta[0][1])}
            p3_drt = {0: load_drt(0)}
            p3_h1d = {0: load_h1ds(0)}

            for g in range(NG):
                gqs = gather_group(g, g1pool, p3_meta[g][0], ag1_out, HID,
                                   False, "g1")
                # prefetch: meta two ahead, inds/drt/diag one ahead
                if g + 2 < NG:
                    p3_meta[g + 2] = load_group_meta(g + 2, idx)
                if g + 1 < NG:
                    p3_inds[g + 1] = build_inds(g + 1, p3_meta[g + 1][1])
                    p3_drt[g + 1] = load_drt(g + 1)
                    p3_h1d[g + 1] = load_h1ds(g + 1)

                inds = p3_inds[g]
                drt = p3_drt[g]
                h1ds = p3_h1d[g]
                for bb in range(G):
                    blk = g * G + bb
                    ps1 = psapool.tile([128, 128], FP, space="PSUM", tag="ps1")
                    nmm = int(nck[g, :, bb].sum())
                    # self-loop diagonal: ps1[:, j] += h1'[slot j] (transpose
                    # via identity matmul; dst dinv applied later via drt)
                    nc.tensor.matmul(out=ps1[:, :], lhsT=h1ds[bb][:, :],
                                     rhs=ident_t[:, :],
                                     start=True, stop=(nmm == 0))
                    done = 0
                    for q in range(Q):
                        s0 = int(s_ck[g, q, bb])
                        for ck in range(int(nck[g, q, bb])):
                            nc.tensor.matmul(
                                out=ps1[:, :],
                                lhsT=gqs[q][:, s0 + ck, :],
                                rhs=inds[(q, bb)][:, ck, :],
                                start=False, stop=(done == nmm - 1))
                            done += 1
                    t1 = midpool.tile([128, 128], FP, tag="t1")
                    nc.vector.tensor_tensor(
                        out=t1[:, :], in0=ps1[:, :],
                        in1=drt[:, bb * 128:(bb + 1) * 128],
                        op=mybir.AluOpType.mult)
                    r1 = midpool.tile([128, 128], BF, tag="r1")
                    nc.scalar.activation(out=r1[:, :], in_=t1[:, :],
                                         func=mybir.ActivationFunctionType.Relu,
                                         bias=b1_t[:, :1])
                    ps2 = psbpool.tile([128, CPAD], FP, space="PSUM", tag="ps2")
                    nc.tensor.matmul(out=ps2[:, :], lhsT=r1[:, :], rhs=w2_t[:, :],
                                     start=True, stop=True)
                    h2p = midpool.tile([128, CPAD], BF, tag="h2p")
                    nc.scalar.activation(out=h2p[:, :], in_=ps2[:, :],
                                         func=mybir.ActivationFunctionType.Copy,
                                         scale=dp_t[:, blk:blk + 1])
                    nc.scalar.dma_start(
                        out=ag2_in[blk * 128:(blk + 1) * 128, :CPAD],
                        in_=h2p[:, :])

            # ---- AG2 ----
            nc.gpsimd.collective_compute(
                "AllGather", mybir.AluOpType.bypass,
                replica_groups=[list(range(W))],
                ins=[ag2_in[:, :]], outs=[ag2_out[:, :]],
            )

            # ================= phase 5: L2 agg + b2 -> out =================
            def load_h2ds(g):
                hs = []
                for bb in range(G):
                    blk = g * G + bb
                    h2d = mpool.tile([128, CPAD], BF, tag="h2d", bufs=16)
                    nc.sync.dma_start(
                        out=h2d[:, :],
                        in_=ag2_in[blk * 128:(blk + 1) * 128, :CPAD])
                    hs.append(h2d)
                return hs

            p5_meta = {0: load_group_meta(0, idx), 1: load_group_meta(1, idx)}
            p5_inds = {0: build_inds(0, p5_meta[0][1])}
            p5_h2d = {0: load_h2ds(0)}

            for g in range(NG):
                gqs = gather_group(g, g2pool, p5_meta[g][0], ag2_out, CPAD,
                                   True, "g2")
                if g + 2 < NG:
                    p5_meta[g + 2] = load_group_meta(g + 2, idx)
                if g + 1 < NG:
                    p5_inds[g + 1] = build_inds(g + 1, p5_meta[g + 1][1])
                    p5_h2d[g + 1] = load_h2ds(g + 1)

                inds = p5_inds[g]
                h2ds = p5_h2d[g]
                for bb in range(G):
                    blk = g * G + bb
                    ps3 = psbpool.tile([128, CPAD], FP, space="PSUM", tag="ps3")
                    nmm = int(nck[g, :, bb].sum())
                    # self-loop diagonal: ps3[slot, :] += h2'[slot]
                    nc.tensor.matmul(out=ps3[:, :], lhsT=ident_t[:, :],
                                     rhs=h2ds[bb][:, :],
                                     start=True, stop=(nmm == 0))
                    done = 0
                    for q in range(Q):
                        s0 = int(s_ck[g, q, bb])
                        for ck in range(int(nck[g, q, bb])):
                            nc.tensor.matmul(
                                out=ps3[:, :],
                                lhsT=inds[(q, bb)][:, ck, :],
                                rhs=gqs[q][:, s0 + ck, :],
                                start=False, stop=(done == nmm - 1))
                            done += 1
                    o3f = midpool.tile([128, CPAD], FP, tag="o3f")
                    nc.scalar.activation(out=o3f[:, :], in_=ps3[:, :],
                                         func=mybir.ActivationFunctionType.Copy,
                                         scale=dp_t[:, blk:blk + 1])
                    o3 = midpool.tile([128, CPAD], FP, tag="o3")
                    nc.vector.tensor_tensor(out=o3[:, :], in0=o3f[:, :],
                                            in1=b2_t[:, :],
                                            op=mybir.AluOpType.add)
                    nc.scalar.dma_start(out=out_s[blk * 128:(blk + 1) * 128, :],
                                        in_=o3[:, :])

    nc.compile()
    return nc


# ======================================================================
# kernel() entry point
# ======================================================================
import os as _os


LAST_EXEC_NS = None
LAST_RES = None


def kernel(x, edge_index, W1, b1, W2, b2):
    """Full-input GCN kernel: shards across 8 NeuronCores internally."""
    global LAST_EXEC_NS, LAST_RES
    import numpy as _np

    trace = bool(int(_os.environ.get("GCN_TRACE", "0")))
    if trace:
        try:
            import sys as _sys
            import types as _types
            from trn_agent_boot.trn_boot import _ntff_profile_via_ctypes
            if "antenv.axon_hooks" not in _sys.modules:
                _hook = _ntff_profile_via_ctypes("/opt/axon/libaxon_pjrt.so")
                _m = _types.ModuleType("antenv.axon_hooks")
                _m.get_axon_ntff_profile_hook = lambda: _hook
                _m.set_axon_ntff_profile_hook = lambda h: None
                _sys.modules["antenv.axon_hooks"] = _m
        except Exception:
            trace = False

    from concourse.bass_utils import run_bass_kernel_spmd

    cfg = Cfg()
    per_core, meta, _ = preprocess(cfg, x, edge_index, W1, b1, W2, b2)
    nc = build(cfg, meta)
    res = run_bass_kernel_spmd(
        nc, per_core, core_ids=list(range(cfg.W)), trace=trace,
    )
    LAST_EXEC_NS = res.exec_time_ns
    LAST_RES = res
    outs = [res.results[c]["out_s"] for c in range(cfg.W)]
    return _np.ascontiguousarray(postprocess(cfg, outs, meta).astype(_np.float32))


# revision 43
# speedup vs baseline: 1.9108x; 1.0095x over previous
"""GCN 2-layer kernel for trn2: host preprocessing + Bass kernel builder.

Math (per GCNConv, PyG-style):
  out = D^-1/2 (A+I) D^-1/2 (X W) + b
Layer1 -> relu -> Layer2.

Device plan (8 cores, SPMD):
  P1: h1' = dinv .* (x_pos @ W1)   (position-ordered; x host-permuted)
  AG1: allgather h1' -> table [NP, HID] bf16 (row = position)
  P3: per group of G=7 dst blocks: quadrant-round-robin batched dma_gathers
      (<=1024 idxs/call to fit the SDMA ring), per-(group,quadrant,block)
      one-hot indicators (dl = blockInGroup*128+slot vs iota, fp16),
      per-block indicator matmuls -> psum [HID, 128] (transposed), self-loop
      diagonal via identity matmul of the local row block, dinv_dst scale
      (DVE), relu+b1 (ACT), @W2 -> [128, 48], dinv scale -> h2'
  AG2: allgather h2' (256B-pitch rows, first 48 cols used) -> [NP, 128] bf16
  P5: same structure with 96B-payload narrow gathers + b2.
Edge streams use exact-max (cross-core) segment padding; only each
(group, quadrant) call stream rounds up to a 128 multiple.
Host: unpermute rows, slice [:N0, :CLS].
"""

from dataclasses import dataclass

import numpy as np

import concourse.bass as bass
import concourse.mybir as mybir
import concourse.tile as tile
from concourse import bacc

FP = mybir.dt.float32
BF = mybir.dt.bfloat16
IND_BUFS = 32


@dataclass
class Cfg:
    N0: int = 100000     # real nodes
    W: int = 8           # cores
    SHARD: int = 12544   # nodes per core (multiple of 128)
    F: int = 512         # in features
    HID: int = 128
    CLS: int = 40
    CPAD: int = 48       # padded class dim (48*2B = 96B gather payload)
    Q: int = 4           # src quadrants (int16 gather index limit)
    G: int = 7           # dst blocks per gather group

    @property
    def NP(self):
        return self.W * self.SHARD

    @property
    def QS(self):
        return self.NP // self.Q

    @property
    def NB(self):
        return self.SHARD // 128  # blocks per core (98)

    @property
    def NG(self):
        return self.NB // self.G  # groups per core (14)


@dataclass
class Meta:
    kq: np.ndarray = None        # [NG, Q, G] chunks per (group, quadrant, block)
    node_of_pos: np.ndarray = None  # [W, SHARD]


def preprocess(cfg: Cfg, x, edge_index, W1, b1, W2, b2):
    N0, W, SHARD, NP, QS = cfg.N0, cfg.W, cfg.SHARD, cfg.NP, cfg.QS
    NB, Q, G, NG = cfg.NB, cfg.Q, cfg.G, cfg.NG
    x = np.asarray(x, dtype=np.float32)
    edge_index = np.asarray(edge_index)
    W1 = np.asarray(W1, np.float32)
    b1 = np.asarray(b1, np.float32)
    W2 = np.asarray(W2, np.float32)
    b2 = np.asarray(b2, np.float32)

    s = edge_index[0].astype(np.int64)
    d = edge_index[1].astype(np.int64)
    loops = np.arange(N0, dtype=np.int64)
    s_all = np.concatenate([s, loops])
    d_all = np.concatenate([d, loops])

    deg = np.bincount(d_all, minlength=NP).astype(np.float64)
    with np.errstate(divide="ignore"):
        dinv = np.where(deg > 0, 1.0 / np.sqrt(deg), 0.0).astype(np.float32)

    # --- degree-balanced position assignment (global serpentine) ---
    nblk = W * NB
    order = np.argsort(-deg[:NP], kind="stable")
    r = np.arange(NP, dtype=np.int64)
    cyc = r % (2 * nblk)
    blk = np.where(cyc < nblk, cyc, 2 * nblk - 1 - cyc)  # serpentine
    slot_ctr = r // (2 * nblk) * 2 + (cyc >= nblk).astype(np.int64)
    pos_of_node = np.empty(NP, dtype=np.int64)
    pos_of_node[order] = (blk % W) * SHARD + (blk // W) * 128 + slot_ctr
    node_of_pos = np.empty(NP, dtype=np.int64)
    node_of_pos[pos_of_node] = np.arange(NP, dtype=np.int64)

    # --- edge routing (self-loops handled as on-device diagonal) ---
    dst_pos = pos_of_node[d]
    src_pos = pos_of_node[s]
    q_e = src_pos // QS

    core_e = dst_pos // SHARD
    blk_e = (dst_pos % SHARD) // 128
    slot_e = dst_pos % 128
    g_e = blk_e // G
    bbg_e = blk_e % G

    # segment key in (core, g, q, bb) order
    key = ((core_e * NG + g_e) * Q + q_e) * G + bbg_e
    nseg = W * NG * Q * G
    counts = np.bincount(key, minlength=nseg).reshape(W, NG, Q, G)
    # exact-max segment lengths (no per-segment 128 rounding); only each
    # (g,q) call stream rounds up to a chunk multiple
    L = counts.max(axis=0).astype(np.int64)  # [NG, Q, G]
    len_gq = L.sum(axis=2)                   # [NG, Q]
    ch_gq_a = np.ceil(len_gq / 128.0).astype(np.int64)
    pad_gq = ch_gq_a * 128
    base = np.zeros(NG * Q, dtype=np.int64)
    base[1:] = np.cumsum(pad_gq.reshape(-1))[:-1]
    base = base.reshape(NG, Q)
    prefL = np.zeros((NG, Q, G + 1), dtype=np.int64)
    prefL[:, :, 1:] = np.cumsum(L, axis=2)
    EPAD = int(pad_gq.sum())
    CT = EPAD // 128  # total chunks

    # per-seg start offset in the padded stream
    seg_off = (base[:, :, None] + prefL[:, :, :G]).reshape(-1)

    # rank of each edge within its (core,g,q,bb) segment
    order = np.argsort(key, kind="stable")
    key_s = key[order]
    seg_start = np.zeros(nseg + 1, dtype=np.int64)
    seg_start[1:] = np.cumsum(counts.reshape(-1))
    rank = np.arange(len(key_s), dtype=np.int64) - seg_start[key_s]
    tgt = seg_off[key_s % (NG * Q * G)] + rank
    core_s = key_s // (NG * Q * G)

    idx_stream = np.zeros((W, EPAD), dtype=np.int16)
    dl_stream = np.full((W, EPAD), -1.0, dtype=np.float32)
    flat = core_s * EPAD + tgt
    idx_stream.reshape(-1)[flat] = (src_pos - q_e * QS)[order].astype(np.int16)
    # dl carries block-in-group * 128 + slot so boundary chunks stay unambiguous
    dlv = (bbg_e * 128 + slot_e).astype(np.float32)
    dl_stream.reshape(-1)[flat] = dlv[order]

    # wrap idx into gather layout: within each (g,q) call's local stream,
    # local index i -> [i % 16, callcol0*8 + i // 16], tiled x8 to 128 parts.
    # call (g,q) spans segments (g,q,0..G-1): local wrap == global wrap as
    # long as call start offsets are multiples of 16 (they are: x128).
    def wrap_idx(stream):
        # stream [W, EPAD]; global position within call cg0*128.. ; since
        # every call boundary is 128-aligned and wrap is (i%16, i//16), a
        # single global wrap per 16 works iff done call-locally. Call starts
        # are 128-aligned => i_local%16 == i_global%16 and
        # i_local//16 == i_global//16 - call0*8. So one global reshape works.
        a = stream.reshape(W, CT, 8, 16)
        wrapped = a.transpose(0, 3, 1, 2).reshape(W, 16, CT * 8)
        return np.tile(wrapped, (1, 8, 1)).astype(np.int16)

    idx_t = wrap_idx(idx_stream)
    dl_t = dl_stream.reshape(W, CT, 128).transpose(0, 2, 1)  # [W,128,CT]

    import ml_dtypes
    bft = ml_dtypes.bfloat16

    dinv_pos = dinv[node_of_pos.reshape(W, SHARD)]  # [W, SHARD] pos-order

    per_core = []
    for c in range(W):
        # host-gathers x rows into position order (pads -> zero rows)
        nop = node_of_pos[c * SHARD:(c + 1) * SHARD]
        xs = np.zeros((SHARD, cfg.F), np.float32)
        real = nop < N0
        xs[real] = x[nop[real]]
        inp = {
            "xT": np.ascontiguousarray(xs.T).astype(bft),          # [F, SHARD]
            "w1": W1.astype(bft),                                  # [F, HID]
            "b1col": b1.reshape(cfg.HID, 1).copy(),                # [HID, 1]
            "w2p": np.pad(W2, ((0, 0), (0, cfg.CPAD - cfg.CLS))).astype(bft),
            "b2rep": np.broadcast_to(
                np.pad(b2, (0, cfg.CPAD - cfg.CLS)), (128, cfg.CPAD)).copy(),
            "iota": np.broadcast_to(
                np.arange(cfg.G * 128, dtype=np.float32),
                (128, cfg.G * 128)).astype(np.float16).copy(),
            "ident": np.eye(128, dtype=np.float32).astype(bft),
            "idx": idx_t[c],
            "dl": dl_t[c].astype(np.float16),
            # pos-order scale: col bb = dinv at positions of block bb
            "dinv_pc": dinv_pos[c].reshape(cfg.NB, 128).T.copy(),  # [128, NB]
            # pos-order row for dst scaling in [HID, slots] orientation
            "dinv_pr": np.broadcast_to(dinv_pos[c], (128, SHARD)).copy(),
        }
        per_core.append(inp)

    meta = Meta(kq=L, node_of_pos=node_of_pos.reshape(W, SHARD))
    return per_core, meta, dinv


def postprocess(cfg: Cfg, outs, meta: Meta):
    """outs: list of [SHARD, CPAD] per core -> [N0, CLS] in node order."""
    res = np.zeros((cfg.NP, cfg.CPAD), np.float32)
    for c in range(cfg.W):
        res[meta.node_of_pos[c]] = outs[c]
    return res[:cfg.N0, :cfg.CLS]


def dma_gather_narrow(nc, out_ap, in_ap, idxs_ap, num_idxs, elem_size,
                      elem_step, queue_num):
    """dma_gather with elem_size_bytes < 256 (payload narrower than the 256B
    row pitch). Mirrors BassGpSimd.dma_gather's non-transpose DRAM path minus
    the %256 payload assert (ucode only requires the *stride* be a multiple
    of 256B; payload is free)."""
    eng = nc.gpsimd
    assert idxs_ap.dtype == mybir.dt.int16
    esz = elem_size * mybir.dt.size(in_ap.dtype)
    assert esz > 0 and esz % 2 == 0
    stride_bytes = elem_step * mybir.dt.size(in_ap.dtype)
    assert stride_bytes % 256 == 0
    stride_bytes_256 = stride_bytes // 256
    assert stride_bytes_256 < 256
    assert in_ap.ap[0][0] == elem_step
    assert in_ap.ap[-1][1] == out_ap.ap[-1][1] == elem_size
    assert out_ap.ap[0][1] * out_ap.ap[1][1] == num_idxs
    _in_ap = eng.lower_ap_dma(in_ap, for_custom_bir_dma=True)
    _idxs_ap = eng.lower_ap(idxs_ap)
    _out_ap = eng.lower_ap(out_ap)
    return eng.add_instruction(
        mybir.InstDMAGatherAnt(
            name=eng.bass.get_next_instruction_name(),
            ins=[*_in_ap, _idxs_ap, eng.lower_val_access(eng.to_reg(num_idxs))],
            outs=[_out_ap],
            transpose=False,
            num_idxs=num_idxs,
            elem_size=elem_size,
            stride_bytes_256=stride_bytes_256,
            gen_mode=0,
            single_packet=False,
            queue_num=queue_num,
            sbuf_tokens_per_rank=0,
            sbuf_free_dim_per_rank=0,
            sbuf_free_dim_pad_per_rank=0,
            sbuf_byte_offset=0,
        )
    )


def build(cfg: Cfg, meta: Meta):
    W, SHARD, NP, F, HID, CPAD = cfg.W, cfg.SHARD, cfg.NP, cfg.F, cfg.HID, cfg.CPAD
    NB, Q, QS, G, NG = cfg.NB, cfg.Q, cfg.QS, cfg.G, cfg.NG
    L = meta.kq  # [NG, Q, G] exact segment lengths
    len_gq = L.sum(axis=2)
    ch_gq = np.ceil(len_gq / 128.0).astype(np.int64)  # chunks per (g,q)
    ch_g = ch_gq.sum(axis=1)
    CT = int(ch_g.sum())
    CHG = int(ch_g.max())
    CHGQ = int(ch_gq.max())
    g_off = np.zeros(NG + 1, dtype=np.int64)
    g_off[1:] = np.cumsum(ch_g)
    prefL = np.zeros((NG, Q, G + 1), dtype=np.int64)
    prefL[:, :, 1:] = np.cumsum(L, axis=2)
    # per (g,q,bb): chunk range [s_ck, e_ck) within the (g,q) stream
    s_ck = prefL[:, :, :G] // 128
    e_ck = -(-prefL[:, :, 1:] // 128)
    e_ck = np.maximum(e_ck, s_ck)  # L==0 -> empty range
    nck = (e_ck - s_ck) * (L > 0)
    NCKMAX = int(nck.max())
    KT = F // 128
    FP16 = mybir.dt.float16

    nc = bacc.Bacc("TRN2", target_bir_lowering=False, debug=False,
                   num_devices=W, num_swdge_queues=4)

    xT = nc.dram_tensor("xT", [F, SHARD], BF, kind="ExternalInput")
    w1 = nc.dram_tensor("w1", [F, HID], BF, kind="ExternalInput")
    b1col = nc.dram_tensor("b1col", [HID, 1], FP, kind="ExternalInput")
    w2p = nc.dram_tensor("w2p", [HID, CPAD], BF, kind="ExternalInput")
    b2rep = nc.dram_tensor("b2rep", [128, CPAD], FP, kind="ExternalInput")
    iota = nc.dram_tensor("iota", [128, G * 128], FP16, kind="ExternalInput")
    ident = nc.dram_tensor("ident", [128, 128], BF, kind="ExternalInput")
    idx = nc.dram_tensor("idx", [128, CT * 8], mybir.dt.int16, kind="ExternalInput")
    dl = nc.dram_tensor("dl", [128, CT], FP16, kind="ExternalInput")
    dinv_pc = nc.dram_tensor("dinv_pc", [128, NB], FP, kind="ExternalInput")
    dinv_pr = nc.dram_tensor("dinv_pr", [128, SHARD], FP, kind="ExternalInput")
    out_s = nc.dram_tensor("out_s", [SHARD, CPAD], FP, kind="ExternalOutput")

    ag1_in = nc.dram_tensor("ag1_in", [SHARD, HID], BF)
    ag1_out = nc.dram_tensor("ag1_out", [NP, HID], BF, addr_space="Shared")
    # L2 table: 256B-pitch rows, only first CPAD cols used
    ag2_in = nc.dram_tensor("ag2_in", [SHARD, 128], BF)
    ag2_out = nc.dram_tensor("ag2_out", [NP, 128], BF, addr_space="Shared")

    qctr = [0]

    def next_q():
        qctr[0] = (qctr[0] + 1) % 4
        return qctr[0]

    with tile.TileContext(nc) as tc:
        with (
            tc.tile_pool(name="const", bufs=1) as cpool,
            tc.tile_pool(name="p1", bufs=4) as p1pool,
            tc.tile_pool(name="meta1", bufs=2) as mpool,
            tc.tile_pool(name="gath1", bufs=2) as g1pool,
            tc.tile_pool(name="gath2", bufs=2) as g2pool,
            tc.tile_pool(name="indp", bufs=IND_BUFS) as ipool,
            tc.tile_pool(name="mid", bufs=3) as midpool,
            tc.tile_pool(name="psa", bufs=2, space="PSUM") as psapool,
            tc.tile_pool(name="psb", bufs=2, space="PSUM") as psbpool,
        ):
            # ---- constants ----
            iota_t = cpool.tile([128, G * 128], FP16)
            nc.sync.dma_start(out=iota_t[:, :], in_=iota[:, :])
            ident_t = cpool.tile([128, 128], BF)
            nc.sync.dma_start(out=ident_t[:, :], in_=ident[:, :])
            b1_t = cpool.tile([HID, 1], FP)
            nc.sync.dma_start(out=b1_t[:, :], in_=b1col[:, :])
            w2_t = cpool.tile([HID, CPAD], BF)
            nc.sync.dma_start(out=w2_t[:, :], in_=w2p[:, :])
            b2_t = cpool.tile([128, CPAD], FP)
            nc.sync.dma_start(out=b2_t[:, :], in_=b2rep[:, :])
            dp_t = cpool.tile([128, NB], FP)
            nc.sync.dma_start(out=dp_t[:, :], in_=dinv_pc[:, :])
            w1k_t = cpool.tile([128, KT, HID], BF)
            for k in range(KT):
                nc.sync.dma_start(out=w1k_t[:, k, :], in_=w1[k * 128:(k + 1) * 128, :])

            # ---- phase 1: h1' = dinv .* (x @ W1) ----
            # load x in 7-block column chunks (few big DMAs, not 392 small)
            TC = 7 * 128
            for ch in range(NB // 7):
                xc = p1pool.tile([128, KT, TC], BF, tag="xc", bufs=2)
                for k in range(KT):
                    nc.sync.dma_start(
                        out=xc[:, k, :],
                        in_=xT[k * 128:(k + 1) * 128, ch * TC:(ch + 1) * TC])
                for tt in range(7):
                    t = ch * 7 + tt
                    psh = psapool.tile([128, HID], FP, space="PSUM", tag="ph1")
                    for k in range(KT):
                        nc.tensor.matmul(
                            out=psh[:, :],
                            lhsT=xc[:, k, tt * 128:(tt + 1) * 128],
                            rhs=w1k_t[:, k, :],
                            start=(k == 0), stop=(k == KT - 1))
                    h1p = p1pool.tile([128, HID], BF, tag="h1p")
                    nc.scalar.activation(out=h1p[:, :], in_=psh[:, :],
                                         func=mybir.ActivationFunctionType.Copy,
                                         scale=dp_t[:, t:t + 1])
                    nc.scalar.dma_start(out=ag1_in[t * 128:(t + 1) * 128, :],
                                        in_=h1p[:, :])

            # ---- AG1 ----
            nc.gpsimd.collective_compute(
                "AllGather", mybir.AluOpType.bypass,
                replica_groups=[list(range(W))],
                ins=[ag1_in[:, :]], outs=[ag1_out[:, :]],
            )

            # ================= phase 3: L1 agg + relu + @W2 =================
            def load_group_meta(g, idx_dram):
                c0, c1 = int(g_off[g]), int(g_off[g + 1])
                ncch = c1 - c0
                ixt = mpool.tile([128, CHG * 8], mybir.dt.int16, tag="ix",
                                 bufs=3)
                nc.sync.dma_start(out=ixt[:, :ncch * 8],
                                  in_=idx_dram[:, c0 * 8:c1 * 8])
                dlt = mpool.tile([128, CHG], FP16, tag="dl", bufs=3)
                nc.sync.dma_start(out=dlt[:, :ncch], in_=dl[:, c0:c1])
                return ixt, dlt

            def build_inds(g, dlt):
                """one indicator tile per (g, q, bb) over its chunk range;
                dl carries bbg*128+slot so rows of other blocks compare to 0"""
                inds = {}
                co = 0
                for q in range(Q):
                    for bb in range(G):
                        n = int(nck[g, q, bb])
                        if n == 0:
                            inds[(q, bb)] = None
                            continue
                        s0 = int(s_ck[g, q, bb])
                        ind = ipool.tile([128, NCKMAX, 128], BF, tag="ind")
                        nc.vector.tensor_tensor(
                            out=ind[:, :n, :],
                            in0=dlt[:, co + s0:co + s0 + n]
                                .to_broadcast([128, n, 128]),
                            in1=iota_t[:, None, bb * 128:(bb + 1) * 128]
                                .to_broadcast([128, n, 128]),
                            op=mybir.AluOpType.is_equal,
                        )
                        inds[(q, bb)] = ind
                    co += int(ch_gq[g, q])
                return inds

            # calls capped at CAPCH chunks (2048 idxs = 128 descs/engine) so a
            # whole call fits the SDMA descriptor ring without Q7 busy-waiting
            CAPCH = 16

            def gather_group(g, pool, ixt, table, width, narrow, tag):
                """q-round-robin sub-calls so all quadrants complete evenly"""
                tiles = [pool.tile([128, CHGQ, width], BF, tag=tag, bufs=8,
                                   name=f"gq{tag}{g}_{q}")
                         for q in range(Q)]
                s = 0
                while True:
                    any_left = False
                    for q in range(Q):
                        co = int(ch_gq[g, :q].sum())
                        nq = int(ch_gq[g, q])
                        if s >= nq:
                            continue
                        any_left = True
                        n = min(CAPCH, nq - s)
                        if narrow:
                            dma_gather_narrow(
                                nc, tiles[q][:, s:s + n, :],
                                table[q * QS:(q + 1) * QS, :width],
                                ixt[:, (co + s) * 8:(co + s + n) * 8],
                                n * 128, width, 128, next_q())
                        else:
                            nc.gpsimd.dma_gather(
                                tiles[q][:, s:s + n, :],
                                table[q * QS:(q + 1) * QS, :],
                                ixt[:, (co + s) * 8:(co + s + n) * 8],
                                n * 128, n * 128, width,
                                single_packet=False, queue_num=next_q(),
                            )
                    if not any_left:
                        break
                    s += CAPCH
                return tiles

            def load_drt(g):
                drt = mpool.tile([128, G * 128], FP, tag="drt", bufs=3)
                nc.sync.dma_start(
                    out=drt[:, :],
                    in_=dinv_pr[:, g * G * 128:(g + 1) * G * 128])
                return drt

            def load_h1ds(g):
                hs = []
                for bb in range(G):
                    blk = g * G + bb
                    h1d = mpool.tile([128, HID], BF, tag="h1d", bufs=16)
                    nc.sync.dma_start(
                        out=h1d[:, :],
                        in_=ag1_in[blk * 128:(blk + 1) * 128, :])
                    hs.append(h1d)
                return hs

            # prologue: two-group lookahead of metadata (flows during AG1)
            p3_meta = {0: load_group_meta(0, idx), 1: load_group_meta(1, idx)}
            p3_inds = {0: build_inds(0, p3_meta[0][1])}
            p3_drt = {0: load_drt(0)}
            p3_h1d = {0: load_h1ds(0)}

            for g in range(NG):
                gqs = gather_group(g, g1pool, p3_meta[g][0], ag1_out, HID,
                                   False, "g1")
                # prefetch: meta two ahead, inds/drt/diag one ahead
                if g + 2 < NG:
                    p3_meta[g + 2] = load_group_meta(g + 2, idx)
                if g + 1 < NG:
                    p3_inds[g + 1] = build_inds(g + 1, p3_meta[g + 1][1])
                    p3_drt[g + 1] = load_drt(g + 1)
                    p3_h1d[g + 1] = load_h1ds(g + 1)

                inds = p3_inds[g]
                drt = p3_drt[g]
                h1ds = p3_h1d[g]
                for bb in range(G):
                    blk = g * G + bb
                    ps1 = psapool.tile([128, 128], FP, space="PSUM", tag="ps1")
                    nmm = int(nck[g, :, bb].sum())
                    # self-loop diagonal: ps1[:, j] += h1'[slot j] (transpose
                    # via identity matmul; dst dinv applied later via drt)
                    nc.tensor.matmul(out=ps1[:, :], lhsT=h1ds[bb][:, :],
                                     rhs=ident_t[:, :],
                                     start=True, stop=(nmm == 0))
                    done = 0
                    for q in range(Q):
                        s0 = int(s_ck[g, q, bb])
                        for ck in range(int(nck[g, q, bb])):
                            nc.tensor.matmul(
                                out=ps1[:, :],
                                lhsT=gqs[q][:, s0 + ck, :],
                                rhs=inds[(q, bb)][:, ck, :],
                                start=False, stop=(done == nmm - 1))
                            done += 1
                    t1 = midpool.tile([128, 128], FP, tag="t1")
                    nc.vector.tensor_tensor(
                        out=t1[:, :], in0=ps1[:, :],
                        in1=drt[:, bb * 128:(bb + 1) * 128],
                        op=mybir.AluOpType.mult)
                    r1 = midpool.tile([128, 128], BF, tag="r1")
                    nc.scalar.activation(out=r1[:, :], in_=t1[:, :],
                                         func=mybir.ActivationFunctionType.Relu,
                                         bias=b1_t[:, :1])
                    ps2 = psbpool.tile([128, CPAD], FP, space="PSUM", tag="ps2")
                    nc.tensor.matmul(out=ps2[:, :], lhsT=r1[:, :], rhs=w2_t[:, :],
                                     start=True, stop=True)
                    h2p = midpool.tile([128, CPAD], BF, tag="h2p")
                    nc.scalar.activation(out=h2p[:, :], in_=ps2[:, :],
                                         func=mybir.ActivationFunctionType.Copy,
                                         scale=dp_t[:, blk:blk + 1])
                    nc.scalar.dma_start(
                        out=ag2_in[blk * 128:(blk + 1) * 128, :CPAD],
                        in_=h2p[:, :])

            # ---- AG2 ----
            nc.gpsimd.collective_compute(
                "AllGather", mybir.AluOpType.bypass,
                replica_groups=[list(range(W))],
                ins=[ag2_in[:, :]], outs=[ag2_out[:, :]],
            )

            # ================= phase 5: L2 agg + b2 -> out =================
            def load_h2ds(g):
                hs = []
                for bb in range(G):
                    blk = g * G + bb
                    h2d = mpool.tile([128, CPAD], BF, tag="h2d", bufs=16)
                    nc.sync.dma_start(
                        out=h2d[:, :],
                        in_=ag2_in[blk * 128:(blk + 1) * 128, :CPAD])
                    hs.append(h2d)
                return hs

            p5_meta = {0: load_group_meta(0, idx), 1: load_group_meta(1, idx)}
            p5_inds = {0: build_inds(0, p5_meta[0][1])}
            p5_h2d = {0: load_h2ds(0)}

            for g in range(NG):
                gqs = gather_group(g, g2pool, p5_meta[g][0], ag2_out, CPAD,
                                   True, "g2")
                if g + 2 < NG:
                    p5_meta[g + 2] = load_group_meta(g + 2, idx)
                if g + 1 < NG:
                    p5_inds[g + 1] = build_inds(g + 1, p5_meta[g + 1][1])
                    p5_h2d[g + 1] = load_h2ds(g + 1)

                inds = p5_inds[g]
                h2ds = p5_h2d[g]
                for bb in range(G):
                    blk = g * G + bb
                    ps3 = psbpool.tile([128, CPAD], FP, space="PSUM", tag="ps3")
                    nmm = int(nck[g, :, bb].sum())
                    # self-loop diagonal: ps3[slot, :] += h2'[slot]
                    nc.tensor.matmul(out=ps3[:, :], lhsT=ident_t[:, :],
                                     rhs=h2ds[bb][:, :],
                                     start=True, stop=(nmm == 0))
                    done = 0
                    for q in range(Q):
                        s0 = int(s_ck[g, q, bb])
                        for ck in range(int(nck[g, q, bb])):
                            nc.tensor.matmul(
                                out=ps3[:, :],
                                lhsT=inds[(q, bb)][:, ck, :],
                                rhs=gqs[q][:, s0 + ck, :],
                                start=False, stop=(done == nmm - 1))
                            done += 1
                    o3f = midpool.tile([128, CPAD], FP, tag="o3f")
                    nc.scalar.activation(out=o3f[:, :], in_=ps3[:, :],
                                         func=mybir.ActivationFunctionType.Copy,
                                         scale=dp_t[:, blk:blk + 1])
                    o3 = midpool.tile([128, CPAD], FP, tag="o3")
                    nc.vector.tensor_tensor(out=o3[:, :], in0=o3f[:, :],
                                            in1=b2_t[:, :],
                                            op=mybir.AluOpType.add)
                    nc.scalar.dma_start(out=out_s[blk * 128:(blk + 1) * 128, :],
                                        in_=o3[:, :])

    nc.compile()
    return nc


# ======================================================================
# kernel() entry point
# ======================================================================
import os as _os


LAST_EXEC_NS = None
LAST_RES = None


def kernel(x, edge_index, W1, b1, W2, b2):
    """Full-input GCN kernel: shards across 8 NeuronCores internally."""
    global LAST_EXEC_NS, LAST_RES
    import numpy as _np

    trace = bool(int(_os.environ.get("GCN_TRACE", "0")))
    if trace:
        try:
            import sys as _sys
            import types as _types
            from trn_agent_boot.trn_boot import _ntff_profile_via_ctypes
            if "antenv.axon_hooks" not in _sys.modules:
                _hook = _ntff_profile_via_ctypes("/opt/axon/libaxon_pjrt.so")
                _m = _types.ModuleType("antenv.axon_hooks")
                _m.get_axon_ntff_profile_hook = lambda: _hook
                _m.set_axon_ntff_profile_hook = lambda h: None
                _sys.modules["antenv.axon_hooks"] = _m
        except Exception:
            trace = False

    from concourse.bass_utils import run_bass_kernel_spmd

    cfg = Cfg()
    per_core, meta, _ = preprocess(cfg, x, edge_index, W1, b1, W2, b2)
    nc = build(cfg, meta)
    res = run_bass_kernel_spmd(
        nc, per_core, core_ids=list(range(cfg.W)), trace=trace,
    )
    LAST_EXEC_NS = res.exec_time_ns
    LAST_RES = res
    outs = [res.results[c]["out_s"] for c in range(cfg.W)]
    return _np.ascontiguousarray(postprocess(cfg, outs, meta).astype(_np.float32))
